# revision 2
# baseline (speedup 1.0000x reference)
"""Kascade reuse attention (sparse tile attention) on 8 TRN2 NeuronCores.

Sharding: data-parallel over batch (2) x tensor-parallel over head groups (4),
one (batch, head-group-of-4) pair per core. Each core computes
partial_out = attn_out(4 heads) @ Wo[rows of those heads]  -> [S, DM]
and the host sums the 4 partials per batch (the "all-reduce after Wo").

Self-contained: hardcodes all shapes from the problem spec.
"""

import numpy as np
from contextlib import ExitStack

import concourse.bass as bass
import concourse.tile as tile
from concourse import bacc, mybir
from concourse import bass_utils

# Problem constants
B, S, DM = 2, 4096, 2048
H, D = 16, 128
TILE, NSEL = 16, 64
K = NSEL * TILE  # 1024 selected keys per head

# Per-core constants
NH = 4           # heads per core
P = 128
DMC = DM // P    # 16 contraction chunks
TOKC = S // 512  # 8 token 512-chunks
KB = K // P      # 8 key blocks per head
QC = S // 512    # 8 query 512-chunks

F32 = mybir.dt.float32
F32R = mybir.dt.float32r
BF16 = mybir.dt.bfloat16
I32 = mybir.dt.int32

MASK_BIG = 1.0e10


def _r(ap):
    return ap


def build_nc():
    nc = bacc.Bacc("TRN2", target_bir_lowering=False, debug=False, num_devices=8)

    xT_d = nc.dram_tensor("xT", [DM, S], BF16, kind="ExternalInput").ap()
    xg_d = nc.dram_tensor("xg", [S, DM], BF16, kind="ExternalInput").ap()
    wq_d = nc.dram_tensor("wq", [DM, NH * D], BF16, kind="ExternalInput").ap()
    wkv_d = nc.dram_tensor("wkv", [DM, NH * 2 * D], BF16, kind="ExternalInput").ap()
    wo_d = nc.dram_tensor("wo", [NH * D, DM], BF16, kind="ExternalInput").ap()
    gidx_d = nc.dram_tensor("gidx", [P, NH * KB], I32, kind="ExternalInput").ap()
    mt_d = nc.dram_tensor("mt", [P, NH * KB * QC], F32, kind="ExternalInput").ap()
    out_d = nc.dram_tensor("out", [S, DM], F32, kind="ExternalOutput").ap()

    # NEFF-embedded constants
    import ml_dtypes
    ident_np = np.eye(P, dtype=ml_dtypes.bfloat16)
    iota_np = np.broadcast_to(np.arange(512, dtype=np.float32), (P, 512)).copy()
    ones_np = np.ones((P, 1), dtype=ml_dtypes.bfloat16)
    oinv_np = np.full((P, 1), 1.0 / K, dtype=ml_dtypes.bfloat16)
    onesr_np = np.ones((1, P), dtype=ml_dtypes.bfloat16)
    ident_d = nc.inline_tensor(ident_np, "ident").ap()
    iota_d = nc.inline_tensor(iota_np, "iota").ap()
    ones_d = nc.inline_tensor(ones_np, "ones").ap()
    oinv_d = nc.inline_tensor(oinv_np, "oinv").ap()
    onesr_d = nc.inline_tensor(onesr_np, "onesr").ap()

    with tile.TileContext(nc) as tc, ExitStack() as ctx:
        emit(ctx, tc,
             xT_d=xT_d, xg_d=xg_d, wq_d=wq_d, wkv_d=wkv_d, wo_d=wo_d,
             gidx_d=gidx_d, mt_d=mt_d, out_d=out_d,
             ident_d=ident_d, iota_d=iota_d, ones_d=ones_d, oinv_d=oinv_d,
             onesr_d=onesr_d)

    nc.compile()
    return nc


def emit(ctx, tc, *, xT_d, xg_d, wq_d, wkv_d, wo_d, gidx_d, mt_d, out_d,
         ident_d, iota_d, ones_d, oinv_d, onesr_d):
    nc = tc.nc
    AL = mybir.AluOpType
    AF = mybir.ActivationFunctionType

    # ---------------- persistent tiles ----------------
    cpool = ctx.enter_context(tc.tile_pool(name="const", bufs=1))
    ident = cpool.tile([P, P], BF16, tag="ident")
    iota = cpool.tile([P, 512], F32, tag="iota")
    ones = cpool.tile([P, 1], BF16, tag="ones")
    oinv = cpool.tile([P, 1], BF16, tag="oinv")
    onesr = cpool.tile([1, P], BF16, tag="onesr")
    gidx = cpool.tile([P, NH * KB], I32, tag="gidx")
    mt = cpool.tile([P, NH * KB * QC], F32, tag="mt")
    nc.sync.dma_start(ident[:], ident_d[:, :])
    nc.sync.dma_start(iota[:], iota_d[:, :])
    nc.sync.dma_start(ones[:], ones_d[:, :])
    nc.sync.dma_start(oinv[:], oinv_d[:, :])
    nc.sync.dma_start(onesr[:], onesr_d[:, :])
    nc.sync.dma_start(gidx[:], gidx_d[:, :])
    nc.sync.dma_start(mt[:], mt_d[:, :])

    qpool = ctx.enter_context(tc.tile_pool(name="qT", bufs=1))
    qT = [qpool.tile([P, S], BF16, tag=f"qT{h}", name=f"qT{h}") for h in range(NH)]

    kvpool = ctx.enter_context(tc.tile_pool(name="kv", bufs=1))
    vsb = [kvpool.tile([P, K], BF16, tag=f"v{h}", name=f"v{h}") for h in range(NH)]
    kT = [kvpool.tile([P, K], BF16, tag=f"kT{h}", name=f"kT{h}") for h in range(NH)]
    vsum = [kvpool.tile([1, D], BF16, tag=f"vsum{h}", name=f"vsum{h}")
            for h in range(NH)]

    # ---------------- phase A: Q projection ----------------
    # qT[h] [d=128, tok] = sum_c wq[c,h].T @ xT[c, tok]
    with tc.tile_pool(name="wqp", bufs=1) as wqp, \
         tc.tile_pool(name="xA", bufs=20) as xA, \
         tc.tile_pool(name="psA", bufs=3, space="PSUM") as psA:
        wq_sb = wqp.tile([P, DMC * NH * D], BF16, tag="wq")
        for c in range(DMC):
            nc.sync.dma_start(wq_sb[:, c * 512:(c + 1) * 512],
                              wq_d[c * P:(c + 1) * P, :])
        for t in range(TOKC):
            xts = []
            for c in range(DMC):
                xt = xA.tile([P, 512], BF16, tag="xA")
                nc.sync.dma_start(xt[:], xT_d[c * P:(c + 1) * P, t * 512:(t + 1) * 512])
                xts.append(xt)
            for h in range(NH):
                ps = psA.tile([P, 512], F32)
                for c in range(DMC):
                    nc.tensor.matmul(
                        ps[:],
                        lhsT=wq_sb[:, c * 512 + h * P: c * 512 + (h + 1) * P],
                        rhs=xts[c][:],
                        start=(c == 0), stop=(c == DMC - 1))
                nc.vector.tensor_copy(qT[h][:, t * 512:(t + 1) * 512], ps[:])

    # ---------------- phase B: gather + sparse K/V projection ----------------
    with tc.tile_pool(name="wkvp", bufs=2) as wkvp, \
         tc.tile_pool(name="gp", bufs=2) as gp, \
         tc.tile_pool(name="tp", bufs=2) as tp, \
         tc.tile_pool(name="ktmp", bufs=2) as ktp, \
         tc.tile_pool(name="psT", bufs=2, space="PSUM") as psT, \
         tc.tile_pool(name="psKV", bufs=2, space="PSUM") as psKV, \
         tc.tile_pool(name="psVS", bufs=2, space="PSUM") as psVS:
        for h in range(NH):
            wkvh = wkvp.tile([P, DMC * 2 * D], BF16, tag="wkv")
            for c in range(DMC):
                nc.sync.dma_start(wkvh[:, c * 256:(c + 1) * 256],
                                  wkv_d[c * P:(c + 1) * P, h * 256:(h + 1) * 256])
            pvs = psVS.tile([1, D], F32)
            for kb in range(KB):
                xg_sb = gp.tile([P, DM], BF16, tag="xg")
                col = h * KB + kb
                nc.gpsimd.indirect_dma_start(
                    out=xg_sb[:], out_offset=None,
                    in_=xg_d[:, :],
                    in_offset=bass.IndirectOffsetOnAxis(ap=gidx[:, col:col + 1], axis=0))
                # transpose 16 [128,128] chunks -> xTs [dm-part, tok]
                xTs = tp.tile([P, DM], BF16, tag="xTs")
                for g in range(4):
                    pst = psT.tile([P, 512], BF16)
                    for cc in range(4):
                        c = g * 4 + cc
                        nc.tensor.transpose(
                            pst[:, cc * P:(cc + 1) * P],
                            xg_sb[:, c * P:(c + 1) * P],
                            ident[:])
                    nc.scalar.copy(xTs[:, g * 512:(g + 1) * 512], pst[:])
                # fused K|V projection: out [tok 128, 256]
                pkv = psKV.tile([P, 2 * D], F32)
                for c in range(DMC):
                    nc.tensor.matmul(
                        pkv[:],
                        lhsT=xTs[:, c * P:(c + 1) * P],
                        rhs=wkvh[:, c * 256:(c + 1) * 256],
                        start=(c == 0), stop=(c == DMC - 1))
                # v part straight to vsb
                nc.vector.tensor_copy(vsb[h][:, kb * P:(kb + 1) * P], pkv[:, D:2 * D])
                # k part -> transpose -> kT
                ktmp = ktp.tile([P, D], BF16, tag="ktmp")
                nc.vector.tensor_copy(ktmp[:], pkv[:, 0:D])
                pst2 = psT.tile([P, 512], BF16)
                nc.tensor.transpose(pst2[:, 0:P], ktmp[:], ident[:])
                nc.vector.tensor_copy(kT[h][:, kb * P:(kb + 1) * P], pst2[:, 0:P])
                # vsum accumulation: [1, D] += ones(1/K).T @ v_kb
                nc.tensor.matmul(
                    pvs[:], lhsT=oinv[:], rhs=vsb[h][:, kb * P:(kb + 1) * P],
                    start=(kb == 0), stop=(kb == KB - 1))
            nc.vector.tensor_copy(vsum[h][:], pvs[:])

    # ---------------- phase C: attention + Wo ----------------
    with tc.tile_pool(name="wop", bufs=1) as wop, \
         tc.tile_pool(name="pp", bufs=KB + 1) as pp, \
         tc.tile_pool(name="capp", bufs=3) as capp, \
         tc.tile_pool(name="lmp", bufs=2) as lmp, \
         tc.tile_pool(name="attnp", bufs=NH) as attnp, \
         tc.tile_pool(name="fixp", bufs=1) as fixp, \
         tc.tile_pool(name="outp", bufs=2) as outp, \
         tc.tile_pool(name="psL", bufs=2, space="PSUM") as psL, \
         tc.tile_pool(name="psO", bufs=2, space="PSUM") as psO, \
         tc.tile_pool(name="psS", bufs=2, space="PSUM") as psS, \
         tc.tile_pool(name="psW", bufs=2, space="PSUM") as psW:
        wo_sb = wop.tile([P, NH * DM], BF16, tag="wo")
        for hh in range(NH):
            nc.sync.dma_start(wo_sb[:, hh * DM:(hh + 1) * DM],
                              wo_d[hh * P:(hh + 1) * P, :])
        for qc in range(QC):
            attn = [attnp.tile([P, 512], BF16, tag="attn", name=f"attn{qc}_{i}") for i in range(NH)]
            for pair in range(NH // 2):
                psum_s = psS.tile([P, 512], F32, tag="ps_s", name=f"psum_s{qc}_{pair}")
                po_pair = []
                for hp in range(2):
                    h = pair * 2 + hp
                    ptiles = []
                    for kb in range(KB):
                        pl = psL.tile([P, 512], F32)
                        nc.tensor.matmul(
                            pl[:],
                            lhsT=kT[h][:, kb * P:(kb + 1) * P],
                            rhs=qT[h][:, qc * 512:(qc + 1) * 512],
                            start=True, stop=True)
                        col = (h * KB + kb) * QC + qc
                        cap = capp.tile([P, 512], F32, tag="cap")
                        nc.gpsimd.tensor_scalar(
                            out=cap[:], in0=iota[:],
                            scalar1=mt[:, col:col + 1], scalar2=MASK_BIG,
                            op0=AL.subtract, op1=AL.mult)
                        lm = lmp.tile([P, 512], F32, tag="lm")
                        nc.vector.tensor_tensor(
                            out=lm[:], in0=pl[:], in1=cap[:], op=AL.min)
                        pt = pp.tile([P, 512], BF16, tag="p")
                        nc.scalar.activation(pt[:], lm[:], AF.Exp)
                        ptiles.append(pt)
                    # key-sums: row at partition 64*hp of the shared bank
                    for kb in range(KB):
                        nc.tensor.matmul(
                            psum_s[64 * hp:64 * hp + 1, :],
                            lhsT=ones[:], rhs=ptiles[kb][:],
                            start=(kb == 0), stop=(kb == KB - 1))
                    # PV: po [d, q] accumulates; group stays open for the fix matmul
                    po = psO.tile([P, 512], F32)
                    for kb in range(KB):
                        nc.tensor.matmul(
                            po[:],
                            lhsT=vsb[h][:, kb * P:(kb + 1) * P],
                            rhs=ptiles[kb][:],
                            start=(kb == 0), stop=False)
                    po_pair.append(po)
                # fix chain for the pair: fix01 = (sums == 0); sums2 = sums + fix01
                fixrow = []
                sumrow = []
                for hp in range(2):
                    srow = psum_s[64 * hp:64 * hp + 1, :]
                    fixf = fixp.tile([1, 512], F32, tag=f"fixf{hp}",
                                     name=f"fixf{qc}_{pair}_{hp}")
                    fixb = fixp.tile([1, 512], BF16, tag=f"fixb{hp}",
                                     name=f"fixb{qc}_{pair}_{hp}")
                    sumb = fixp.tile([1, 512], BF16, tag=f"sumb{hp}",
                                     name=f"sumb{qc}_{pair}_{hp}")
                    nc.vector.tensor_scalar(
                        out=fixf[:], in0=srow, scalar1=0.0, scalar2=None,
                        op0=AL.is_equal)
                    nc.vector.tensor_copy(fixb[:], fixf[:])
                    nc.vector.tensor_tensor(
                        out=sumb[:], in0=srow, in1=fixf[:], op=AL.add)
                    fixrow.append(fixb[:])
                    sumrow.append(sumb[:])
                for hp in range(2):
                    h = pair * 2 + hp
                    # rank-1 all-masked fixup: po += vsum[h].T @ fix01[hp]
                    nc.tensor.matmul(
                        po_pair[hp][:],
                        lhsT=vsum[h][:],
                        rhs=fixrow[hp],
                        start=False, stop=True)
                    # broadcast sums row across partitions via PE outer product,
                    # then reciprocal on the broadcast (fp32)
                    pbt = psS.tile([P, 512], F32, tag="ps_s", name=f"pbt{qc}_{pair}_{hp}")
                    nc.tensor.matmul(
                        pbt[:], lhsT=onesr[:], rhs=sumrow[hp],
                        start=True, stop=True)
                    rb = capp.tile([P, 512], F32, tag="cap", name=f"rb{qc}_{pair}_{hp}")
                    nc.scalar.copy(rb[:], pbt[:])
                    rbr = capp.tile([P, 512], F32, tag="cap", name=f"rbr{qc}_{pair}_{hp}")
                    rbs = capp.tile([P, 512], F32, tag="cap", name=f"rbs{qc}_{pair}_{hp}")
                    nc.vector.reciprocal_approx_accurate(
                        out=rbr[:], in_=rb[:], scratch=rbs[:])
                    # normalize + evict
                    nc.vector.tensor_tensor(
                        out=attn[h][:], in0=po_pair[hp][:],
                        in1=rbr[:], op=AL.mult)
            # Wo: out[tok, dm] partial
            for tb in range(4):
                for n in range(4):
                    pw = psW.tile([P, 512], F32)
                    for hh in range(NH):
                        nc.tensor.matmul(
                            pw[:],
                            lhsT=attn[hh][:, tb * P:(tb + 1) * P],
                            rhs=wo_sb[:, hh * DM + n * 512: hh * DM + (n + 1) * 512],
                            start=(hh == 0), stop=(hh == NH - 1))
                    osb = outp.tile([P, 512], F32, tag="osb")
                    nc.scalar.copy(osb[:], pw[:])
                    nc.sync.dma_start(
                        out_d[qc * 512 + tb * P: qc * 512 + (tb + 1) * P,
                              n * 512:(n + 1) * 512],
                        osb[:])


def make_in_maps(x, Wq, Wk, Wv, Wo, anchor_indices):
    scale = 1.0 / np.sqrt(np.float32(D))
    x = np.asarray(x, dtype=np.float32)
    Wq = np.asarray(Wq, dtype=np.float32)
    Wk = np.asarray(Wk, dtype=np.float32)
    Wv = np.asarray(Wv, dtype=np.float32)
    Wo = np.asarray(Wo, dtype=np.float32)
    anchor = np.asarray(anchor_indices)

    in_maps = []
    for core in range(8):
        b, hg = core // 4, core % 4
        heads = [4 * hg + h for h in range(NH)]
        import ml_dtypes
        bf = ml_dtypes.bfloat16
        xT_b = np.ascontiguousarray(x[b].T).astype(bf)
        xg_b = np.ascontiguousarray(x[b]).astype(bf)
        wq_c = np.ascontiguousarray(Wq[:, 4 * hg * D:(4 * hg + 4) * D] * scale).astype(bf)
        wkv_c = np.empty((DM, NH * 2 * D), dtype=bf)
        for h, gh in enumerate(heads):
            wkv_c[:, h * 256:h * 256 + D] = Wk[:, gh * D:(gh + 1) * D]
            wkv_c[:, h * 256 + D:(h + 1) * 256] = Wv[:, gh * D:(gh + 1) * D]
        wo_c = np.ascontiguousarray(Wo[4 * hg * D:(4 * hg + 4) * D, :]).astype(bf)

        tiles = anchor[b, 4 * hg:4 * hg + 4, :].astype(np.int64).copy()
        tiles[:, -1] = (S - 1) // TILE
        tok = (tiles[:, :, None] * TILE
               + np.arange(TILE, dtype=np.int64)[None, None, :]).reshape(NH, K)

        gidx_c = np.empty((P, NH * KB), dtype=np.int32)
        mt_c = np.empty((P, NH * KB * QC), dtype=np.float32)
        for h in range(NH):
            for kb in range(KB):
                seg = tok[h, kb * P:(kb + 1) * P]
                gidx_c[:, h * KB + kb] = seg
                for qc in range(QC):
                    mt_c[:, (h * KB + kb) * QC + qc] = seg - 512.0 * qc - 0.5

        in_maps.append({
            "xT": xT_b, "xg": xg_b, "wq": wq_c, "wkv": wkv_c, "wo": wo_c,
            "gidx": gidx_c, "mt": mt_c,
        })
    return in_maps


_NC_CACHE = {}


def get_nc():
    if "nc" not in _NC_CACHE:
        _NC_CACHE["nc"] = build_nc()
    return _NC_CACHE["nc"]


def _ensure_axon_hook_stub():
    # The NTFF profile hook module is absent in some containers; stub it so
    # run_bass_kernel_spmd(trace=True) degrades to a no-trace run.
    import sys, types
    try:
        from antenv import axon_hooks  # noqa: F401
    except ImportError:
        mod = types.ModuleType("antenv.axon_hooks")
        mod.get_axon_ntff_profile_hook = lambda: None
        sys.modules["antenv.axon_hooks"] = mod
        import antenv
        antenv.axon_hooks = mod


def kernel(x, Wq, Wk, Wv, Wo, anchor_indices, _trace=False, _trace_dir=None):
    in_maps = make_in_maps(x, Wq, Wk, Wv, Wo, anchor_indices)
    nc = get_nc()
    if _trace:
        _ensure_axon_hook_stub()
    res = bass_utils.run_bass_kernel_spmd(
        nc, in_maps, core_ids=list(range(8)), trace=_trace, tmpdir=_trace_dir)
    out = np.zeros((B, S, DM), dtype=np.float32)
    for core in range(8):
        out[core // 4] += res.results[core]["out"]
    if _trace:
        kernel.last_exec_time_ns = res.exec_time_ns
        kernel.last_results = res
    return out



# revision 19
# speedup vs baseline: 3.8975x; 3.8975x over previous
"""Kascade reuse attention (sparse tile attention) on 8 TRN2 NeuronCores.

Sharding: data-parallel over batch (2) x tensor-parallel over head groups (4),
one (batch, head-group-of-4) pair per core. Each core computes
partial_out = attn_out(4 heads) @ Wo[rows of those heads]  -> [S, DM]
and the host sums the 4 partials per batch (the "all-reduce after Wo").

Key design points (v2):
- Selected K/V tokens are gathered from DRAM with dma_gather(transpose=True),
  which lands x^T tiles [dm-chunk, token] directly in SBUF — no PE transposes.
- K is projected straight into kT [d, tok] layout (lhsT = Wk chunk); V is
  projected into [tok, d] layout (lhsT = gathered x^T chunk).
- Tiles are sorted per head on the host; (head, key-block, query-chunk) pairs
  that are fully masked on ALL cores are skipped at compile time, pairs that
  need no mask on ANY core skip the mask ops. The causal mask is a 0/1
  multiply on DVE (was: tensor_scalar on GpSimd — the old bottleneck).
- Softmax denominators come from a DVE pre-add of the prob tiles plus a single
  ones-matmul per (qc, head).
- Output partials are written in bf16 and summed on the host in f32.

Self-contained: hardcodes all shapes from the problem spec.
"""

import numpy as np
from contextlib import ExitStack

import concourse.bass as bass
import concourse.tile as tile
from concourse import bacc, mybir
from concourse import bass_utils

# Problem constants
B, S, DM = 2, 4096, 2048
H, D = 16, 128
TILE, NSEL = 16, 64
K = NSEL * TILE  # 1024 selected keys per head

# Per-core constants
NH = 4           # heads per core
P = 128
DMC = DM // P    # 16 contraction chunks
KB = K // P      # 8 key blocks per head
QC = S // 512    # 8 query 512-chunks
TOKC = S // 512  # 8 token 512-chunks (phase A)

F32 = mybir.dt.float32
BF16 = mybir.dt.bfloat16
I16 = mybir.dt.int16


# ---------------------------------------------------------------------------
# classification: which (qc, h, b) logits blocks exist / need masking
# ---------------------------------------------------------------------------

def _sorted_tokens(anchor):
    """tok[core, h_local, 1024] sorted ascending, with the forced last tile."""
    anchor = np.asarray(anchor)
    tok = np.empty((8, NH, K), dtype=np.int64)
    for core in range(8):
        b, hg = core // 4, core % 4
        for h in range(NH):
            tiles = anchor[b, 4 * hg + h].astype(np.int64).copy()
            tiles[-1] = (S - 1) // TILE
            tiles = np.sort(tiles)
            tok[core, h] = (tiles[:, None] * TILE + np.arange(TILE)).reshape(-1)
    return tok


def classify(anchor):
    """Union classification across the 8 cores sharing one NEFF.

    Returns (kept, partial_order, fixqc):
      kept[(qc, h)] = tuple of key-blocks b to compute
      partial_order = tuple of (qc, h, b) triples needing a mask, in the
        canonical order that also indexes the mt table columns
      fixqc = tuple of query chunks that may contain all-masked query rows
    """
    tok = _sorted_tokens(anchor)
    mn = tok[:, :, ::P].min(axis=0)            # [NH, KB] min over cores of block-min
    mx = tok[:, :, P - 1::P].max(axis=0)       # [NH, KB] max over cores of block-max
    kept = {}
    partial_order = []
    for qc in range(QC):
        for h in range(NH):
            bl = []
            for b in range(KB):
                if mn[h, b] > qc * 512 + 511:
                    continue                    # fully masked on every core
                bl.append(b)
                if mx[h, b] > qc * 512:
                    partial_order.append((qc, h, b))
            kept[(qc, h)] = tuple(bl)
    maxtok0 = int(tok[:, :, 0].max())
    fixqc = tuple(qc for qc in range(QC) if qc * 512 < maxtok0)
    return kept, tuple(partial_order), fixqc


# ---------------------------------------------------------------------------
# kernel build
# ---------------------------------------------------------------------------

def build_nc(cls, dbg=False):
    kept, partial_order, fixqc = cls
    npart = max(1, len(partial_order))

    nc = bacc.Bacc("TRN2", target_bir_lowering=False, debug=False, num_devices=8)

    xT_d = nc.dram_tensor("xT", [DM, S], BF16, kind="ExternalInput").ap()
    xg_d = nc.dram_tensor("xg", [S, DM], BF16, kind="ExternalInput").ap()
    wq_d = nc.dram_tensor("wq", [DM, NH * D], BF16, kind="ExternalInput").ap()
    wk_d = nc.dram_tensor("wk", [DM, NH * D], BF16, kind="ExternalInput").ap()
    wv_d = nc.dram_tensor("wv", [DM, NH * D], BF16, kind="ExternalInput").ap()
    wo_d = nc.dram_tensor("wo", [NH * D, DM], BF16, kind="ExternalInput").ap()
    gidx_d = nc.dram_tensor("gidx", [P, NH * (K // 16)], I16, kind="ExternalInput").ap()
    mt_d = nc.dram_tensor("mt", [P, npart], F32, kind="ExternalInput").ap()
    out_d = nc.dram_tensor("out", [S, DM], BF16, kind="ExternalOutput").ap()
    dbg_d = (nc.dram_tensor("dbg", [P, 16384], BF16, kind="ExternalOutput").ap()
             if dbg else None)

    # NEFF-embedded constants
    import ml_dtypes
    bf = ml_dtypes.bfloat16
    iota_np = np.broadcast_to(np.arange(512, dtype=np.float32), (P, 512)).copy()
    ones_np = np.ones((P, 1), dtype=bf)
    oinv_np = np.full((P, 1), 1.0 / K, dtype=bf)
    onesr_np = np.ones((1, P), dtype=bf)
    onesrow_np = np.ones((1, 512), dtype=bf)
    iota_d = nc.inline_tensor(iota_np, "iota").ap()
    ones_d = nc.inline_tensor(ones_np, "ones").ap()
    oinv_d = nc.inline_tensor(oinv_np, "oinv").ap()
    onesr_d = nc.inline_tensor(onesr_np, "onesr").ap()
    onesrow_d = nc.inline_tensor(onesrow_np, "onesrow").ap()

    with tile.TileContext(nc) as tc, ExitStack() as ctx:
        emit(ctx, tc, cls,
             xT_d=xT_d, xg_d=xg_d, wq_d=wq_d, wk_d=wk_d, wv_d=wv_d, wo_d=wo_d,
             gidx_d=gidx_d, mt_d=mt_d, out_d=out_d, dbg_d=dbg_d,
             iota_d=iota_d, ones_d=ones_d, oinv_d=oinv_d, onesr_d=onesr_d,
             onesrow_d=onesrow_d)

    nc.compile()
    return nc


def emit(ctx, tc, cls, *, xT_d, xg_d, wq_d, wk_d, wv_d, wo_d, gidx_d, mt_d,
         out_d, dbg_d=None, iota_d, ones_d, oinv_d, onesr_d, onesrow_d):
    kept, partial_order, fixqc = cls
    pcol = {t: i for i, t in enumerate(partial_order)}
    fixqc = set(fixqc)
    nc = tc.nc
    AL = mybir.AluOpType
    AF = mybir.ActivationFunctionType

    # ---------------- persistent tiles ----------------
    cpool = ctx.enter_context(tc.tile_pool(name="const", bufs=1))
    iota = cpool.tile([P, 512], F32, tag="iota")
    ones = cpool.tile([P, 1], BF16, tag="ones")
    oinv = cpool.tile([P, 1], BF16, tag="oinv")
    onesr = cpool.tile([1, P], BF16, tag="onesr")
    onesrow = cpool.tile([1, 512], BF16, tag="onesrow")
    gidx = cpool.tile([P, NH * (K // 16)], I16, tag="gidx")
    mt = cpool.tile([P, max(1, len(partial_order))], F32, tag="mt")
    nc.sync.dma_start(iota[:], iota_d[:, :])
    nc.sync.dma_start(ones[:], ones_d[:, :])
    nc.sync.dma_start(oinv[:], oinv_d[:, :])
    nc.sync.dma_start(onesr[:], onesr_d[:, :])
    nc.sync.dma_start(onesrow[:], onesrow_d[:, :])
    nc.sync.dma_start(gidx[:], gidx_d[:, :])
    nc.sync.dma_start(mt[:], mt_d[:, :])

    qpool = ctx.enter_context(tc.tile_pool(name="qT", bufs=1))
    qT = [qpool.tile([P, S], BF16, tag=f"qT{h}", name=f"qT{h}") for h in range(NH)]

    kvpool = ctx.enter_context(tc.tile_pool(name="kv", bufs=1))
    kT = [kvpool.tile([P, K], BF16, tag=f"kT{h}", name=f"kT{h}") for h in range(NH)]
    vsb = [kvpool.tile([P, K], BF16, tag=f"v{h}", name=f"v{h}") for h in range(NH)]
    vsum = [kvpool.tile([1, D], BF16, tag=f"vsum{h}", name=f"vsum{h}")
            for h in range(NH)]

    # ---------------- gathers (software-pipelined) ----------------
    # xts[h, hf][p, c, i] = x[tok[h, hf*512+i], c*128+p]  for 512 tokens
    halves = [(h, hf) for h in range(NH) for hf in range(2)]
    bstack = ExitStack()  # closed after phase B to free the gather buffers
    xtsp = bstack.enter_context(tc.tile_pool(name="xts", bufs=3))
    gath = {}

    def issue_gather(h, hf):
        t = xtsp.tile([P, DMC, 512], BF16, tag="xts", name=f"xts{h}_{hf}")
        col = h * 64 + hf * 32
        nc.gpsimd.dma_gather(
            t[:], xg_d[:, :], gidx[:, col:col + 32], 512, 512, DM,
            transpose=True)
        gath[(h, hf)] = t

    for h, hf in halves[:3]:
        issue_gather(h, hf)

    # ---------------- phase A: Q projection ----------------
    # qT[h] [d=128, tok] = sum_c wq[c,h].T @ xT[c, tok]
    with tc.tile_pool(name="wqp", bufs=1) as wqp, \
         tc.tile_pool(name="xA", bufs=20) as xA, \
         tc.tile_pool(name="psA", bufs=3, space="PSUM") as psA:
        wq_sb = wqp.tile([P, DMC * NH * D], BF16, tag="wq")
        for c in range(DMC):
            nc.sync.dma_start(wq_sb[:, c * 512:(c + 1) * 512],
                              wq_d[c * P:(c + 1) * P, :])
        for t in range(TOKC):
            xts = []
            for c in range(DMC):
                xt = xA.tile([P, 512], BF16, tag="xA")
                nc.sync.dma_start(xt[:], xT_d[c * P:(c + 1) * P, t * 512:(t + 1) * 512])
                xts.append(xt)
            for h in range(NH):
                ps = psA.tile([P, 512], F32)
                for c in range(DMC):
                    nc.tensor.matmul(
                        ps[:],
                        lhsT=wq_sb[:, c * 512 + h * P: c * 512 + (h + 1) * P],
                        rhs=xts[c][:],
                        start=(c == 0), stop=(c == DMC - 1))
                nc.vector.tensor_copy(qT[h][:, t * 512:(t + 1) * 512], ps[:])

    # ---------------- phase B: sparse K/V projection ----------------
    with tc.tile_pool(name="wkp", bufs=1) as wkp, \
         tc.tile_pool(name="wvp", bufs=1) as wvp, \
         tc.tile_pool(name="psK", bufs=2, space="PSUM") as psK, \
         tc.tile_pool(name="psV", bufs=2, space="PSUM") as psV, \
         tc.tile_pool(name="psVS", bufs=2, space="PSUM") as psVS:
        wk_sb = wkp.tile([P, DMC * NH * D], BF16, tag="wk")
        wv_sb = wvp.tile([P, DMC * NH * D], BF16, tag="wv")
        for c in range(DMC):
            nc.sync.dma_start(wk_sb[:, c * 512:(c + 1) * 512],
                              wk_d[c * P:(c + 1) * P, :])
            nc.sync.dma_start(wv_sb[:, c * 512:(c + 1) * 512],
                              wv_d[c * P:(c + 1) * P, :])
        gi = 3
        for h in range(NH):
            for hf in range(2):
                xts = gath[(h, hf)]
                # K: kT[h][:, hf*512:(hf+1)*512] = sum_c wk_c.T @ xts_c
                psk = psK.tile([P, 512], F32)
                for c in range(DMC):
                    nc.tensor.matmul(
                        psk[:],
                        lhsT=wk_sb[:, c * 512 + h * P: c * 512 + (h + 1) * P],
                        rhs=xts[:, c, :],
                        start=(c == 0), stop=(c == DMC - 1))
                nc.vector.tensor_copy(kT[h][:, hf * 512:(hf + 1) * 512], psk[:])
                # V: vsb[h][:, (hf*4+kbl)*128 + d] = gathered_x @ wv  [tok, d]
                psv = psV.tile([P, 512], F32)
                for kbl in range(4):
                    for c in range(DMC):
                        nc.tensor.matmul(
                            psv[:, kbl * P:(kbl + 1) * P],
                            lhsT=xts[:, c, kbl * P:(kbl + 1) * P],
                            rhs=wv_sb[:, c * 512 + h * P: c * 512 + (h + 1) * P],
                            start=(c == 0), stop=(c == DMC - 1))
                nc.vector.tensor_copy(vsb[h][:, hf * 512:(hf + 1) * 512], psv[:])
                if gi < len(halves):
                    issue_gather(*halves[gi])
                    gi += 1
            # vsum[h] = (1/K) * sum over all selected tokens of v
            pvs = psVS.tile([1, D], F32)
            for kb in range(KB):
                nc.tensor.matmul(
                    pvs[:], lhsT=oinv[:], rhs=vsb[h][:, kb * P:(kb + 1) * P],
                    start=(kb == 0), stop=(kb == KB - 1))
            nc.vector.tensor_copy(vsum[h][:], pvs[:])

    if dbg_d is not None:
        nc.sync.dma_start(dbg_d[:, 0:4096], qT[0][:, :])
        nc.sync.dma_start(dbg_d[:, 4096:5120], kT[0][:, :])
        nc.sync.dma_start(dbg_d[:, 5120:6144], vsb[0][:, :])
        nc.sync.dma_start(dbg_d[0:1, 6144:6272], vsum[0][:, :])

    bstack.close()

    # ---------------- phase C: attention + Wo ----------------
    with tc.tile_pool(name="wop", bufs=1) as wop, \
         tc.tile_pool(name="ptp", bufs=22) as ptp, \
         tc.tile_pool(name="indp", bufs=8) as indp, \
         tc.tile_pool(name="attnp", bufs=8) as attnp, \
         tc.tile_pool(name="rowp", bufs=2) as rowp, \
         tc.tile_pool(name="rbcp", bufs=3) as rbcp, \
         tc.tile_pool(name="outp", bufs=4) as outp, \
         tc.tile_pool(name="psL", bufs=3, space="PSUM") as psL, \
         tc.tile_pool(name="psO", bufs=2, space="PSUM") as psO, \
         tc.tile_pool(name="psSum", bufs=1, space="PSUM") as psSum, \
         tc.tile_pool(name="psW", bufs=2, space="PSUM") as psW:
        wo_sb = wop.tile([P, NH * DM], BF16, tag="wo")
        for hh in range(NH):
            nc.sync.dma_start(wo_sb[:, hh * DM:(hh + 1) * DM],
                              wo_d[hh * P:(hh + 1) * P, :])

        for qc in range(QC):
            sums = psSum.tile([P, 512], F32, tag="sums", name=f"sums{qc}")
            attn_t = {}
            po_t = {}
            use_t = {}

            def stage1(h):
                """logits + exp + mask for head h"""
                bl = kept[(qc, h)]
                uses = []
                for b in bl:
                    pl = psL.tile([P, 512], F32)
                    nc.tensor.matmul(
                        pl[:],
                        lhsT=kT[h][:, b * P:(b + 1) * P],
                        rhs=qT[h][:, qc * 512:(qc + 1) * 512],
                        start=True, stop=True)
                    pt = ptp.tile([P, 512], BF16, tag="pt")
                    nc.scalar.activation(pt[:], pl[:], AF.Exp)
                    if (qc, h, b) in pcol:
                        col = pcol[(qc, h, b)]
                        ind = indp.tile([P, 512], BF16, tag="ind")
                        nc.vector.tensor_scalar(
                            out=ind[:], in0=iota[:], scalar1=mt[:, col:col + 1],
                            scalar2=None, op0=AL.is_ge)
                        ptm = ptp.tile([P, 512], BF16, tag="pt")
                        nc.vector.tensor_tensor(
                            out=ptm[:], in0=pt[:], in1=ind[:], op=AL.mult)
                        uses.append(ptm)
                        if dbg_d is not None and qc == 4 and h == 0 and b == bl[-1]:
                            nc.sync.dma_start(dbg_d[:, 6784:7296], ind[:])
                    else:
                        uses.append(pt)
                    if dbg_d is not None and qc == 4 and h == 0 and b == bl[-1]:
                        nc.sync.dma_start(dbg_d[:, 6272:6784], uses[-1][:])
                use_t[h] = uses

            def stage2(h):
                """PV + denominator + fix for head h"""
                bl = kept[(qc, h)]
                uses = use_t[h]
                po = psO.tile([P, 512], F32, tag="po", name=f"po{qc}_{h}")
                po_t[h] = po
                if not bl:
                    nc.tensor.matmul(po[:], lhsT=vsum[h][:], rhs=onesrow[:],
                                     start=True, stop=True)
                    return
                need_fix = qc in fixqc
                for j, b in enumerate(bl):
                    nc.tensor.matmul(
                        po[:],
                        lhsT=vsb[h][:, b * P:(b + 1) * P],
                        rhs=uses[j][:],
                        start=(j == 0),
                        stop=(j == len(bl) - 1 and not need_fix))
                # denominator: DVE pre-add then one ones-matmul
                if len(uses) == 1:
                    padd = uses[0]
                else:
                    padd = ptp.tile([P, 512], BF16, tag="pt")
                    nc.vector.tensor_tensor(
                        out=padd[:], in0=uses[0][:], in1=uses[1][:], op=AL.add)
                    for u in uses[2:]:
                        nc.vector.tensor_tensor(
                            out=padd[:], in0=padd[:], in1=u[:], op=AL.add)
                srow = sums[32 * h:32 * h + 1, :]
                nc.tensor.matmul(srow, lhsT=ones[:], rhs=padd[:],
                                 start=True, stop=True,
                                 tile_position=(0, 32 * h))
                if need_fix:
                    fixf = rowp.tile([1, 512], F32, tag="fixf")
                    nc.vector.tensor_scalar(
                        out=fixf[:], in0=srow, scalar1=0.0, scalar2=None,
                        op0=AL.is_equal)
                    fixb = rowp.tile([1, 512], BF16, tag="fixb")
                    nc.vector.tensor_copy(fixb[:], fixf[:])
                    sumb = rowp.tile([1, 512], F32, tag="sumb")
                    nc.vector.tensor_tensor(
                        out=sumb[:], in0=srow, in1=fixf[:], op=AL.add)
                    nc.tensor.matmul(po[:], lhsT=vsum[h][:], rhs=fixb[:],
                                     start=False, stop=True)
                    sin = sumb[:]
                else:
                    # reciprocal_approx_accurate mis-reads PSUM rows at
                    # partition offset != 0 — stage through a partition-0
                    # SBUF row first.
                    scp = rowp.tile([1, 512], F32, tag="scp")
                    nc.vector.tensor_copy(scp[:], srow)
                    sin = scp[:]
                rsc = rowp.tile([1, 512], F32, tag="rsc")
                rss = rowp.tile([1, 512], F32, tag="rss")
                nc.vector.reciprocal_approx_accurate(
                    out=rsc[:], in_=sin, scratch=rss[:])
                rrow = rowp.tile([1, 512], BF16, tag="rrow")
                nc.vector.tensor_copy(rrow[:], rsc[:])
                if dbg_d is not None and qc == 1:
                    nc.sync.dma_start(dbg_d[0:1, 8320 + 512 * h:8320 + 512 * (h + 1)],
                                      rrow[:])
                use_t[h] = rrow  # stash for stage3

            def stage3(h):
                """broadcast reciprocal + normalize head h"""
                po = po_t[h]
                at = attnp.tile([P, 512], BF16, tag="attn", name=f"at{qc}_{h}")
                if not kept[(qc, h)]:
                    nc.vector.tensor_copy(at[:], po[:])
                    attn_t[h] = at
                    return
                rrow = use_t[h]
                pbt = psW.tile([P, 512], F32, tag="pw", name=f"pbt{qc}_{h}")
                nc.tensor.matmul(pbt[:], lhsT=onesr[:], rhs=rrow[:],
                                 start=True, stop=True)
                rbc = rbcp.tile([P, 512], BF16, tag="rbc")
                nc.scalar.copy(rbc[:], pbt[:])
                nc.vector.tensor_tensor(
                    out=at[:], in0=po[:], in1=rbc[:], op=AL.mult)
                if dbg_d is not None and qc == 4 and h == 0:
                    nc.sync.dma_start(dbg_d[:, 7296:7808], at[:])
                if dbg_d is not None and qc == 1:
                    nc.sync.dma_start(dbg_d[:, 10368 + 512 * h:10368 + 512 * (h + 1)],
                                      at[:])
                attn_t[h] = at

            # 3-stage head pipeline
            plan = [(stage1, 0), (stage1, 1), (stage2, 0), (stage1, 2),
                    (stage2, 1), (stage3, 0), (stage1, 3), (stage2, 2),
                    (stage3, 1), (stage2, 3), (stage3, 2), (stage3, 3)]
            for fn, h in plan:
                fn(h)

            if dbg_d is not None and qc == 1:
                sdump = outp.tile([P, 512], BF16, tag="osb")
                nc.vector.tensor_copy(sdump[:], sums[:])
                nc.sync.dma_start(dbg_d[:, 12416:12928], sdump[:])

            # Wo: out[tok, dm] partial
            for tb in range(4):
                for n in range(4):
                    pw = psW.tile([P, 512], F32, tag="pw",
                                  name=f"pw{qc}_{tb}_{n}")
                    for hh in range(NH):
                        nc.tensor.matmul(
                            pw[:],
                            lhsT=attn_t[hh][:, tb * P:(tb + 1) * P],
                            rhs=wo_sb[:, hh * DM + n * 512: hh * DM + (n + 1) * 512],
                            start=(hh == 0), stop=(hh == NH - 1))
                    osb = outp.tile([P, 512], BF16, tag="osb")
                    if (tb + n) % 2 == 0:
                        nc.vector.tensor_copy(osb[:], pw[:])
                    else:
                        nc.scalar.copy(osb[:], pw[:])
                    nc.sync.dma_start(
                        out_d[qc * 512 + tb * P: qc * 512 + (tb + 1) * P,
                              n * 512:(n + 1) * 512],
                        osb[:])


# ---------------------------------------------------------------------------
# host side
# ---------------------------------------------------------------------------

def make_in_maps(x, Wq, Wk, Wv, Wo, anchor_indices, cls):
    import ml_dtypes
    bf = ml_dtypes.bfloat16
    kept, partial_order, fixqc = cls
    scale = 1.0 / np.sqrt(np.float32(D))
    x = np.asarray(x, dtype=np.float32)
    Wq = np.asarray(Wq, dtype=np.float32)
    Wk = np.asarray(Wk, dtype=np.float32)
    Wv = np.asarray(Wv, dtype=np.float32)
    Wo = np.asarray(Wo, dtype=np.float32)
    tok = _sorted_tokens(anchor_indices)

    xT_b = [np.ascontiguousarray(x[b].T).astype(bf) for b in range(B)]
    xg_b = [np.ascontiguousarray(x[b]).astype(bf) for b in range(B)]

    in_maps = []
    for core in range(8):
        b, hg = core // 4, core % 4
        sl = slice(4 * hg * D, (4 * hg + 4) * D)
        wq_c = np.ascontiguousarray(Wq[:, sl] * scale).astype(bf)
        wk_c = np.ascontiguousarray(Wk[:, sl]).astype(bf)
        wv_c = np.ascontiguousarray(Wv[:, sl]).astype(bf)
        wo_c = np.ascontiguousarray(Wo[sl, :]).astype(bf)

        # gather indices: per (h, half) group of 512, entry i wrapped to
        # [i % 16, col + i // 16], replicated across the 8 gpsimd stripes
        gidx_c = np.zeros((16, NH * (K // 16)), dtype=np.int16)
        for h in range(NH):
            for hf in range(2):
                seg = tok[core, h, hf * 512:(hf + 1) * 512].astype(np.int16)
                gidx_c[:, h * 64 + hf * 32: h * 64 + (hf + 1) * 32] = \
                    seg.reshape(32, 16).T
        gidx_c = np.tile(gidx_c, (8, 1))

        npart = max(1, len(partial_order))
        mt_c = np.zeros((P, npart), dtype=np.float32)
        for i, (qc, h, bb) in enumerate(partial_order):
            mt_c[:, i] = tok[core, h, bb * P:(bb + 1) * P] - 512.0 * qc - 0.5

        in_maps.append({
            "xT": xT_b[b], "xg": xg_b[b], "wq": wq_c, "wk": wk_c, "wv": wv_c,
            "wo": wo_c, "gidx": gidx_c, "mt": mt_c,
        })
    return in_maps


_NC_CACHE = {}


def get_nc(cls):
    key = (tuple(sorted(cls[0].items())), cls[1], cls[2])
    if key not in _NC_CACHE:
        _NC_CACHE[key] = build_nc(cls)
    return _NC_CACHE[key]


def _ensure_axon_hook_stub():
    # The NTFF profile hook module is absent in some containers; stub it so
    # run_bass_kernel_spmd(trace=True) degrades to a no-trace run.
    import sys, types
    try:
        from antenv import axon_hooks  # noqa: F401
    except ImportError:
        mod = types.ModuleType("antenv.axon_hooks")
        mod.get_axon_ntff_profile_hook = lambda: None
        sys.modules["antenv.axon_hooks"] = mod
        import antenv
        antenv.axon_hooks = mod


def kernel(x, Wq, Wk, Wv, Wo, anchor_indices, _trace=False, _trace_dir=None):
    cls = classify(anchor_indices)
    in_maps = make_in_maps(x, Wq, Wk, Wv, Wo, anchor_indices, cls)
    nc = get_nc(cls)
    if _trace:
        _ensure_axon_hook_stub()
    res = bass_utils.run_bass_kernel_spmd(
        nc, in_maps, core_ids=list(range(8)), trace=_trace, tmpdir=_trace_dir)
    out = np.zeros((B, S, DM), dtype=np.float32)
    for core in range(8):
        out[core // 4] += res.results[core]["out"].astype(np.float32)
    if _trace:
        kernel.last_exec_time_ns = res.exec_time_ns
        kernel.last_results = res
    return out


# revision 23
# speedup vs baseline: 4.4974x; 1.1539x over previous
"""Kascade reuse attention (sparse tile attention) on 8 TRN2 NeuronCores.

Sharding: data-parallel over batch (2) x tensor-parallel over head groups (4),
one (batch, head-group-of-4) pair per core. Each core computes
partial_out = attn_out(4 heads) @ Wo[rows of those heads]  -> [S, DM]
and the host sums the 4 partials per batch (the "all-reduce after Wo").

Key design points (v2):
- Selected K/V tokens are gathered from DRAM with dma_gather(transpose=True),
  which lands x^T tiles [dm-chunk, token] directly in SBUF — no PE transposes.
- K is projected straight into kT [d, tok] layout (lhsT = Wk chunk); V is
  projected into [tok, d] layout (lhsT = gathered x^T chunk).
- Tiles are sorted per head on the host; (head, key-block, query-chunk) pairs
  that are fully masked on ALL cores are skipped at compile time, pairs that
  need no mask on ANY core skip the mask ops. The causal mask is a 0/1
  multiply on DVE (was: tensor_scalar on GpSimd — the old bottleneck).
- Softmax denominators come from a DVE pre-add of the prob tiles plus a single
  ones-matmul per (qc, head).
- Output partials are written in bf16 and summed on the host in f32.

Self-contained: hardcodes all shapes from the problem spec.
"""

import numpy as np
from contextlib import ExitStack

import concourse.bass as bass
import concourse.tile as tile
from concourse import bacc, mybir
from concourse import bass_utils

# Problem constants
B, S, DM = 2, 4096, 2048
H, D = 16, 128
TILE, NSEL = 16, 64
K = NSEL * TILE  # 1024 selected keys per head

# Per-core constants
NH = 4           # heads per core
P = 128
DMC = DM // P    # 16 contraction chunks
KB = K // P      # 8 key blocks per head
QC = S // 512    # 8 query 512-chunks
TOKC = S // 512  # 8 token 512-chunks (phase A)

F32 = mybir.dt.float32
BF16 = mybir.dt.bfloat16
I16 = mybir.dt.int16


# ---------------------------------------------------------------------------
# classification: which (qc, h, b) logits blocks exist / need masking
# ---------------------------------------------------------------------------

def _sorted_tokens(anchor):
    """tok[core, h_local, 1024] sorted ascending, with the forced last tile."""
    anchor = np.asarray(anchor)
    tok = np.empty((8, NH, K), dtype=np.int64)
    for core in range(8):
        b, hg = core // 4, core % 4
        for h in range(NH):
            tiles = anchor[b, 4 * hg + h].astype(np.int64).copy()
            tiles[-1] = (S - 1) // TILE
            tiles = np.sort(tiles)
            tok[core, h] = (tiles[:, None] * TILE + np.arange(TILE)).reshape(-1)
    return tok


def classify(anchor):
    """Union classification across the 8 cores sharing one NEFF.

    Returns (kept, partial_order, fixqc):
      kept[(qc, h)] = tuple of key-blocks b to compute
      partial_order = tuple of (qc, h, b) triples needing a mask, in the
        canonical order that also indexes the mt table columns
      fixqc = tuple of query chunks that may contain all-masked query rows
    """
    tok = _sorted_tokens(anchor)
    mn = tok[:, :, ::P].min(axis=0)            # [NH, KB] min over cores of block-min
    mx = tok[:, :, P - 1::P].max(axis=0)       # [NH, KB] max over cores of block-max
    kept = {}
    partial_order = []
    for qc in range(QC):
        for h in range(NH):
            bl = []
            for b in range(KB):
                if mn[h, b] > qc * 512 + 511:
                    continue                    # fully masked on every core
                bl.append(b)
                if mx[h, b] > qc * 512:
                    partial_order.append((qc, h, b))
            kept[(qc, h)] = tuple(bl)
    maxtok0 = int(tok[:, :, 0].max())
    fixqc = tuple(qc for qc in range(QC) if qc * 512 < maxtok0)
    return kept, tuple(partial_order), fixqc


# ---------------------------------------------------------------------------
# kernel build
# ---------------------------------------------------------------------------

def build_nc(cls, dbg=False):
    kept, partial_order, fixqc = cls
    npart = max(1, len(partial_order))

    nc = bacc.Bacc("TRN2", target_bir_lowering=False, debug=False, num_devices=8)

    xT_d = nc.dram_tensor("xT", [DM, S], BF16, kind="ExternalInput").ap()
    xg_d = nc.dram_tensor("xg", [S, DM], BF16, kind="ExternalInput").ap()
    wq_d = nc.dram_tensor("wq", [DM, NH * D], BF16, kind="ExternalInput").ap()
    wk_d = nc.dram_tensor("wk", [DM, NH * D], BF16, kind="ExternalInput").ap()
    wv_d = nc.dram_tensor("wv", [DM, NH * D], BF16, kind="ExternalInput").ap()
    wo_d = nc.dram_tensor("wo", [NH * D, DM], BF16, kind="ExternalInput").ap()
    gidx_d = nc.dram_tensor("gidx", [P, NH * (K // 16)], I16, kind="ExternalInput").ap()
    mt_d = nc.dram_tensor("mt", [P, npart], F32, kind="ExternalInput").ap()
    out_d = nc.dram_tensor("out", [S, DM], BF16, kind="ExternalOutput").ap()
    dbg_d = (nc.dram_tensor("dbg", [P, 16384], BF16, kind="ExternalOutput").ap()
             if dbg else None)

    # NEFF-embedded constants
    import ml_dtypes
    bf = ml_dtypes.bfloat16
    iota_np = np.broadcast_to(np.arange(512, dtype=np.float32), (P, 512)).copy()
    ones_np = np.ones((P, 1), dtype=bf)
    oinv_np = np.full((P, 1), 1.0 / K, dtype=bf)
    onesr_np = np.ones((1, P), dtype=bf)
    onesrow_np = np.ones((1, 512), dtype=bf)
    iota_d = nc.inline_tensor(iota_np, "iota").ap()
    ones_d = nc.inline_tensor(ones_np, "ones").ap()
    oinv_d = nc.inline_tensor(oinv_np, "oinv").ap()
    onesr_d = nc.inline_tensor(onesr_np, "onesr").ap()
    onesrow_d = nc.inline_tensor(onesrow_np, "onesrow").ap()

    with tile.TileContext(nc) as tc, ExitStack() as ctx:
        emit(ctx, tc, cls,
             xT_d=xT_d, xg_d=xg_d, wq_d=wq_d, wk_d=wk_d, wv_d=wv_d, wo_d=wo_d,
             gidx_d=gidx_d, mt_d=mt_d, out_d=out_d, dbg_d=dbg_d,
             iota_d=iota_d, ones_d=ones_d, oinv_d=oinv_d, onesr_d=onesr_d,
             onesrow_d=onesrow_d)

    nc.compile()
    return nc


def emit(ctx, tc, cls, *, xT_d, xg_d, wq_d, wk_d, wv_d, wo_d, gidx_d, mt_d,
         out_d, dbg_d=None, iota_d, ones_d, oinv_d, onesr_d, onesrow_d):
    kept, partial_order, fixqc = cls
    pcol = {t: i for i, t in enumerate(partial_order)}
    fixqc = set(fixqc)
    nc = tc.nc
    AL = mybir.AluOpType
    AF = mybir.ActivationFunctionType

    # ---------------- persistent tiles ----------------
    cpool = ctx.enter_context(tc.tile_pool(name="const", bufs=1))
    iota = cpool.tile([P, 512], F32, tag="iota")
    ones = cpool.tile([P, 1], BF16, tag="ones")
    oinv = cpool.tile([P, 1], BF16, tag="oinv")
    onesr = cpool.tile([1, P], BF16, tag="onesr")
    onesrow = cpool.tile([1, 512], BF16, tag="onesrow")
    gidx = cpool.tile([P, NH * (K // 16)], I16, tag="gidx")
    mt = cpool.tile([P, max(1, len(partial_order))], F32, tag="mt")
    nc.sync.dma_start(iota[:], iota_d[:, :])
    nc.sync.dma_start(ones[:], ones_d[:, :])
    nc.sync.dma_start(oinv[:], oinv_d[:, :])
    nc.sync.dma_start(onesr[:], onesr_d[:, :])
    nc.sync.dma_start(onesrow[:], onesrow_d[:, :])
    nc.sync.dma_start(gidx[:], gidx_d[:, :])
    nc.sync.dma_start(mt[:], mt_d[:, :])

    qpool = ctx.enter_context(tc.tile_pool(name="qT", bufs=1))
    qT = [qpool.tile([P, S], BF16, tag=f"qT{h}", name=f"qT{h}") for h in range(NH)]

    kvpool = ctx.enter_context(tc.tile_pool(name="kv", bufs=1))
    kT = [kvpool.tile([P, K], BF16, tag=f"kT{h}", name=f"kT{h}") for h in range(NH)]
    vsb = [kvpool.tile([P, K], BF16, tag=f"v{h}", name=f"v{h}") for h in range(NH)]
    vsum = [kvpool.tile([1, D], BF16, tag=f"vsum{h}", name=f"vsum{h}")
            for h in range(NH)]

    # ---------------- phases A+B interleaved ----------------
    # A: qT[h] [d=128, tok] = sum_c wq[c,h].T @ xT[c, tok], 8 token chunks.
    # B: per (h, half): K into kT layout directly, V into vsb layout, fed by
    #    dma_gather(transpose=True) tiles. B half i is emitted after A chunk
    #    i+1 so gathers have 2 chunks of PE time to land.
    halves = [(h, hf) for h in range(NH) for hf in range(2)]
    bstack = ExitStack()  # closed after phase B to free the gather buffers
    xtsp = bstack.enter_context(tc.tile_pool(name="xts", bufs=5))
    gath = {}

    def issue_gather(h, hf):
        t = xtsp.tile([P, DMC, 512], BF16, tag="xts", name=f"xts{h}_{hf}")
        col = h * 64 + hf * 32
        nc.gpsimd.dma_gather(
            t[:], xg_d[:, :], gidx[:, col:col + 32], 512, 512, DM,
            transpose=True)
        gath[(h, hf)] = t

    abstack = ExitStack()
    wqp = abstack.enter_context(tc.tile_pool(name="wqp", bufs=1))
    wkp = abstack.enter_context(tc.tile_pool(name="wkp", bufs=1))
    wvp = abstack.enter_context(tc.tile_pool(name="wvp", bufs=1))
    xA = abstack.enter_context(tc.tile_pool(name="xA", bufs=18))
    psA = abstack.enter_context(tc.tile_pool(name="psA", bufs=2, space="PSUM"))
    psK = abstack.enter_context(tc.tile_pool(name="psK", bufs=2, space="PSUM"))
    psV = abstack.enter_context(tc.tile_pool(name="psV", bufs=2, space="PSUM"))
    psVS = abstack.enter_context(tc.tile_pool(name="psVS", bufs=1, space="PSUM"))

    wq_sb = wqp.tile([P, DMC * NH * D], BF16, tag="wq")
    wk_sb = wkp.tile([P, DMC * NH * D], BF16, tag="wk")
    wv_sb = wvp.tile([P, DMC * NH * D], BF16, tag="wv")
    # startup DMA priority: wq, first x chunk, first gathers, wk/wv
    for c in range(DMC):
        nc.sync.dma_start(wq_sb[:, c * 512:(c + 1) * 512],
                          wq_d[c * P:(c + 1) * P, :])
    xA_tiles = {}

    def emit_xa_dmas(t):
        tiles = []
        for c in range(DMC):
            xt = xA.tile([P, 512], BF16, tag="xA")
            nc.sync.dma_start(xt[:], xT_d[c * P:(c + 1) * P, t * 512:(t + 1) * 512])
            tiles.append(xt)
        xA_tiles[t] = tiles

    emit_xa_dmas(0)
    for h, hf in halves[:5]:
        issue_gather(h, hf)
    for c in range(DMC):
        nc.sync.dma_start(wk_sb[:, c * 512:(c + 1) * 512],
                          wk_d[c * P:(c + 1) * P, :])
        nc.sync.dma_start(wv_sb[:, c * 512:(c + 1) * 512],
                          wv_d[c * P:(c + 1) * P, :])

    def emit_a_chunk(t):
        xts = xA_tiles.pop(t)
        for h in range(NH):
            ps = psA.tile([P, 512], F32)
            for c in range(DMC):
                nc.tensor.matmul(
                    ps[:],
                    lhsT=wq_sb[:, c * 512 + h * P: c * 512 + (h + 1) * P],
                    rhs=xts[c][:],
                    start=(c == 0), stop=(c == DMC - 1))
            nc.vector.tensor_copy(qT[h][:, t * 512:(t + 1) * 512], ps[:])

    gi = 5

    def emit_b_half(i):
        nonlocal gi
        h, hf = halves[i]
        xts = gath[(h, hf)]
        # K: kT[h][:, hf*512:(hf+1)*512] = sum_c wk_c.T @ xts_c
        psk = psK.tile([P, 512], F32)
        for c in range(DMC):
            nc.tensor.matmul(
                psk[:],
                lhsT=wk_sb[:, c * 512 + h * P: c * 512 + (h + 1) * P],
                rhs=xts[:, c, :],
                start=(c == 0), stop=(c == DMC - 1))
        nc.vector.tensor_copy(kT[h][:, hf * 512:(hf + 1) * 512], psk[:])
        # V: vsb[h][:, (hf*4+kbl)*128 + d] = gathered_x @ wv  [tok, d]
        psv = psV.tile([P, 512], F32)
        for kbl in range(4):
            for c in range(DMC):
                nc.tensor.matmul(
                    psv[:, kbl * P:(kbl + 1) * P],
                    lhsT=xts[:, c, kbl * P:(kbl + 1) * P],
                    rhs=wv_sb[:, c * 512 + h * P: c * 512 + (h + 1) * P],
                    start=(c == 0), stop=(c == DMC - 1))
        nc.vector.tensor_copy(vsb[h][:, hf * 512:(hf + 1) * 512], psv[:])
        if gi < len(halves):
            issue_gather(*halves[gi])
            gi += 1
        if hf == 1:
            # vsum[h] = (1/K) * sum over all selected tokens of v
            pvs = psVS.tile([1, D], F32)
            for kb in range(KB):
                nc.tensor.matmul(
                    pvs[:], lhsT=oinv[:], rhs=vsb[h][:, kb * P:(kb + 1) * P],
                    start=(kb == 0), stop=(kb == KB - 1))
            nc.vector.tensor_copy(vsum[h][:], pvs[:])

    for t in range(TOKC):
        emit_a_chunk(t)
        if t + 1 < TOKC:
            emit_xa_dmas(t + 1)
        if t >= 1:
            emit_b_half(t - 1)
    emit_b_half(6)
    emit_b_half(7)
    abstack.close()

    if dbg_d is not None:
        nc.sync.dma_start(dbg_d[:, 0:4096], qT[0][:, :])
        nc.sync.dma_start(dbg_d[:, 4096:5120], kT[0][:, :])
        nc.sync.dma_start(dbg_d[:, 5120:6144], vsb[0][:, :])
        nc.sync.dma_start(dbg_d[0:1, 6144:6272], vsum[0][:, :])

    bstack.close()

    # ---------------- phase C: attention + Wo ----------------
    with tc.tile_pool(name="wop", bufs=1) as wop, \
         tc.tile_pool(name="ptp", bufs=22) as ptp, \
         tc.tile_pool(name="indp", bufs=8) as indp, \
         tc.tile_pool(name="attnp", bufs=8) as attnp, \
         tc.tile_pool(name="rowp", bufs=2) as rowp, \
         tc.tile_pool(name="rbcp", bufs=3) as rbcp, \
         tc.tile_pool(name="outp", bufs=4) as outp, \
         tc.tile_pool(name="psL", bufs=3, space="PSUM") as psL, \
         tc.tile_pool(name="psO", bufs=2, space="PSUM") as psO, \
         tc.tile_pool(name="psSum", bufs=1, space="PSUM") as psSum, \
         tc.tile_pool(name="psW", bufs=2, space="PSUM") as psW:
        wo_sb = wop.tile([P, NH * DM], BF16, tag="wo")
        for hh in range(NH):
            nc.sync.dma_start(wo_sb[:, hh * DM:(hh + 1) * DM],
                              wo_d[hh * P:(hh + 1) * P, :])

        pending_wo = [None]

        def emit_wo(qc, attn_t):
            for tb in range(4):
                for n in range(4):
                    pw = psW.tile([P, 512], F32, tag="pw",
                                  name=f"pw{qc}_{tb}_{n}")
                    for hh in range(NH):
                        nc.tensor.matmul(
                            pw[:],
                            lhsT=attn_t[hh][:, tb * P:(tb + 1) * P],
                            rhs=wo_sb[:, hh * DM + n * 512: hh * DM + (n + 1) * 512],
                            start=(hh == 0), stop=(hh == NH - 1))
                    osb = outp.tile([P, 512], BF16, tag="osb")
                    if (tb + n) % 2 == 0:
                        nc.vector.tensor_copy(osb[:], pw[:])
                    else:
                        nc.scalar.copy(osb[:], pw[:])
                    nc.sync.dma_start(
                        out_d[qc * 512 + tb * P: qc * 512 + (tb + 1) * P,
                              n * 512:(n + 1) * 512],
                        osb[:])

        for qc in range(QC):
            sums = psSum.tile([P, 512], F32, tag="sums", name=f"sums{qc}")
            attn_t = {}
            po_t = {}
            use_t = {}
            padd_t = {}

            def stage1(h):
                """logits + exp + mask + incremental denominator pre-add"""
                bl = kept[(qc, h)]
                uses = []
                padd = None
                for b in bl:
                    pl = psL.tile([P, 512], F32)
                    nc.tensor.matmul(
                        pl[:],
                        lhsT=kT[h][:, b * P:(b + 1) * P],
                        rhs=qT[h][:, qc * 512:(qc + 1) * 512],
                        start=True, stop=True)
                    pt = ptp.tile([P, 512], BF16, tag="pt")
                    nc.scalar.activation(pt[:], pl[:], AF.Exp)
                    if (qc, h, b) in pcol:
                        col = pcol[(qc, h, b)]
                        ind = indp.tile([P, 512], BF16, tag="ind")
                        nc.vector.tensor_scalar(
                            out=ind[:], in0=iota[:], scalar1=mt[:, col:col + 1],
                            scalar2=None, op0=AL.is_ge)
                        ptm = ptp.tile([P, 512], BF16, tag="pt")
                        nc.vector.tensor_tensor(
                            out=ptm[:], in0=pt[:], in1=ind[:], op=AL.mult)
                        uses.append(ptm)
                        if dbg_d is not None and qc == 4 and h == 0 and b == bl[-1]:
                            nc.sync.dma_start(dbg_d[:, 6784:7296], ind[:])
                    else:
                        uses.append(pt)
                    if dbg_d is not None and qc == 4 and h == 0 and b == bl[-1]:
                        nc.sync.dma_start(dbg_d[:, 6272:6784], uses[-1][:])
                    # incremental pre-add for the softmax denominator
                    if len(uses) == 2:
                        padd = ptp.tile([P, 512], BF16, tag="pt")
                        nc.vector.tensor_tensor(
                            out=padd[:], in0=uses[0][:], in1=uses[1][:],
                            op=AL.add)
                    elif len(uses) > 2:
                        nc.vector.tensor_tensor(
                            out=padd[:], in0=padd[:], in1=uses[-1][:],
                            op=AL.add)
                use_t[h] = uses
                padd_t[h] = padd if padd is not None else (
                    uses[0] if uses else None)

            def stage2(h):
                """PV + denominator matmul + fix for head h"""
                bl = kept[(qc, h)]
                uses = use_t[h]
                po = psO.tile([P, 512], F32, tag="po", name=f"po{qc}_{h}")
                po_t[h] = po
                if not bl:
                    nc.tensor.matmul(po[:], lhsT=vsum[h][:], rhs=onesrow[:],
                                     start=True, stop=True)
                    return
                need_fix = qc in fixqc
                for j, b in enumerate(bl):
                    nc.tensor.matmul(
                        po[:],
                        lhsT=vsb[h][:, b * P:(b + 1) * P],
                        rhs=uses[j][:],
                        start=(j == 0),
                        stop=(j == len(bl) - 1 and not need_fix))
                srow = sums[32 * h:32 * h + 1, :]
                nc.tensor.matmul(srow, lhsT=ones[:], rhs=padd_t[h][:],
                                 start=True, stop=True,
                                 tile_position=(0, 32 * h))
                if need_fix:
                    fixf = rowp.tile([1, 512], F32, tag="fixf")
                    nc.vector.tensor_scalar(
                        out=fixf[:], in0=srow, scalar1=0.0, scalar2=None,
                        op0=AL.is_equal)
                    fixb = rowp.tile([1, 512], BF16, tag="fixb")
                    nc.vector.tensor_copy(fixb[:], fixf[:])
                    sumb = rowp.tile([1, 512], F32, tag="sumb")
                    nc.vector.tensor_tensor(
                        out=sumb[:], in0=srow, in1=fixf[:], op=AL.add)
                    nc.tensor.matmul(po[:], lhsT=vsum[h][:], rhs=fixb[:],
                                     start=False, stop=True)
                    sin = sumb[:]
                else:
                    # reciprocal_approx_accurate mis-reads PSUM rows at
                    # partition offset != 0 — stage through a partition-0
                    # SBUF row first.
                    scp = rowp.tile([1, 512], F32, tag="scp")
                    nc.vector.tensor_copy(scp[:], srow)
                    sin = scp[:]
                rsc = rowp.tile([1, 512], F32, tag="rsc")
                rss = rowp.tile([1, 512], F32, tag="rss")
                nc.vector.reciprocal_approx_accurate(
                    out=rsc[:], in_=sin, scratch=rss[:])
                rrow = rowp.tile([1, 512], BF16, tag="rrow")
                nc.vector.tensor_copy(rrow[:], rsc[:])
                if dbg_d is not None and qc == 1:
                    nc.sync.dma_start(dbg_d[0:1, 8320 + 512 * h:8320 + 512 * (h + 1)],
                                      rrow[:])
                use_t[h] = rrow  # stash for stage3

            def stage3(h):
                """broadcast reciprocal + normalize head h"""
                po = po_t[h]
                at = attnp.tile([P, 512], BF16, tag="attn", name=f"at{qc}_{h}")
                if not kept[(qc, h)]:
                    nc.vector.tensor_copy(at[:], po[:])
                    attn_t[h] = at
                    return
                rrow = use_t[h]
                pbt = psW.tile([P, 512], F32, tag="pw", name=f"pbt{qc}_{h}")
                nc.tensor.matmul(pbt[:], lhsT=onesr[:], rhs=rrow[:],
                                 start=True, stop=True)
                rbc = rbcp.tile([P, 512], BF16, tag="rbc")
                nc.scalar.copy(rbc[:], pbt[:])
                nc.vector.tensor_tensor(
                    out=at[:], in0=po[:], in1=rbc[:], op=AL.mult)
                if dbg_d is not None and qc == 4 and h == 0:
                    nc.sync.dma_start(dbg_d[:, 7296:7808], at[:])
                if dbg_d is not None and qc == 1:
                    nc.sync.dma_start(dbg_d[:, 10368 + 512 * h:10368 + 512 * (h + 1)],
                                      at[:])
                attn_t[h] = at

            # 3-stage head pipeline; previous qc's Wo is emitted after this
            # qc's first two logit blocks so its attn inputs have slack.
            stage1(0)
            stage1(1)
            if pending_wo[0] is not None:
                pending_wo[0]()
                pending_wo[0] = None
            for fn, h in [(stage2, 0), (stage1, 2), (stage2, 1), (stage3, 0),
                          (stage1, 3), (stage2, 2), (stage3, 1), (stage2, 3),
                          (stage3, 2), (stage3, 3)]:
                fn(h)

            if dbg_d is not None and qc == 1:
                sdump = outp.tile([P, 512], BF16, tag="osb")
                nc.vector.tensor_copy(sdump[:], sums[:])
                nc.sync.dma_start(dbg_d[:, 12416:12928], sdump[:])

            pending_wo[0] = (lambda qc=qc, attn_t=attn_t: emit_wo(qc, attn_t))

        pending_wo[0]()


# ---------------------------------------------------------------------------
# host side
# ---------------------------------------------------------------------------

def make_in_maps(x, Wq, Wk, Wv, Wo, anchor_indices, cls):
    import ml_dtypes
    bf = ml_dtypes.bfloat16
    kept, partial_order, fixqc = cls
    scale = 1.0 / np.sqrt(np.float32(D))
    x = np.asarray(x, dtype=np.float32)
    Wq = np.asarray(Wq, dtype=np.float32)
    Wk = np.asarray(Wk, dtype=np.float32)
    Wv = np.asarray(Wv, dtype=np.float32)
    Wo = np.asarray(Wo, dtype=np.float32)
    tok = _sorted_tokens(anchor_indices)

    xT_b = [np.ascontiguousarray(x[b].T).astype(bf) for b in range(B)]
    xg_b = [np.ascontiguousarray(x[b]).astype(bf) for b in range(B)]

    in_maps = []
    for core in range(8):
        b, hg = core // 4, core % 4
        sl = slice(4 * hg * D, (4 * hg + 4) * D)
        wq_c = np.ascontiguousarray(Wq[:, sl] * scale).astype(bf)
        wk_c = np.ascontiguousarray(Wk[:, sl]).astype(bf)
        wv_c = np.ascontiguousarray(Wv[:, sl]).astype(bf)
        wo_c = np.ascontiguousarray(Wo[sl, :]).astype(bf)

        # gather indices: per (h, half) group of 512, entry i wrapped to
        # [i % 16, col + i // 16], replicated across the 8 gpsimd stripes
        gidx_c = np.zeros((16, NH * (K // 16)), dtype=np.int16)
        for h in range(NH):
            for hf in range(2):
                seg = tok[core, h, hf * 512:(hf + 1) * 512].astype(np.int16)
                gidx_c[:, h * 64 + hf * 32: h * 64 + (hf + 1) * 32] = \
                    seg.reshape(32, 16).T
        gidx_c = np.tile(gidx_c, (8, 1))

        npart = max(1, len(partial_order))
        mt_c = np.zeros((P, npart), dtype=np.float32)
        for i, (qc, h, bb) in enumerate(partial_order):
            mt_c[:, i] = tok[core, h, bb * P:(bb + 1) * P] - 512.0 * qc - 0.5

        in_maps.append({
            "xT": xT_b[b], "xg": xg_b[b], "wq": wq_c, "wk": wk_c, "wv": wv_c,
            "wo": wo_c, "gidx": gidx_c, "mt": mt_c,
        })
    return in_maps


_NC_CACHE = {}


def get_nc(cls):
    key = (tuple(sorted(cls[0].items())), cls[1], cls[2])
    if key not in _NC_CACHE:
        _NC_CACHE[key] = build_nc(cls)
    return _NC_CACHE[key]


def _ensure_axon_hook_stub():
    # The NTFF profile hook module is absent in some containers; stub it so
    # run_bass_kernel_spmd(trace=True) degrades to a no-trace run.
    import sys, types
    try:
        from antenv import axon_hooks  # noqa: F401
    except ImportError:
        mod = types.ModuleType("antenv.axon_hooks")
        mod.get_axon_ntff_profile_hook = lambda: None
        sys.modules["antenv.axon_hooks"] = mod
        import antenv
        antenv.axon_hooks = mod


def kernel(x, Wq, Wk, Wv, Wo, anchor_indices, _trace=False, _trace_dir=None):
    cls = classify(anchor_indices)
    in_maps = make_in_maps(x, Wq, Wk, Wv, Wo, anchor_indices, cls)
    nc = get_nc(cls)
    if _trace:
        _ensure_axon_hook_stub()
    res = bass_utils.run_bass_kernel_spmd(
        nc, in_maps, core_ids=list(range(8)), trace=_trace, tmpdir=_trace_dir)
    out = np.zeros((B, S, DM), dtype=np.float32)
    for core in range(8):
        out[core // 4] += res.results[core]["out"].astype(np.float32)
    if _trace:
        kernel.last_exec_time_ns = res.exec_time_ns
        kernel.last_results = res
    return out


# revision 30
# speedup vs baseline: 4.7008x; 1.0452x over previous
"""Kascade reuse attention (sparse tile attention) on 8 TRN2 NeuronCores.

Sharding: data-parallel over batch (2) x tensor-parallel over head groups (4),
one (batch, head-group-of-4) pair per core. Each core computes
partial_out = attn_out(4 heads) @ Wo[rows of those heads]  -> [S, DM]
and the host sums the 4 partials per batch (the "all-reduce after Wo").

Key design points (v2):
- Selected K/V tokens are gathered from DRAM with dma_gather(transpose=True),
  which lands x^T tiles [dm-chunk, token] directly in SBUF — no PE transposes.
- K is projected straight into kT [d, tok] layout (lhsT = Wk chunk); V is
  projected into [tok, d] layout (lhsT = gathered x^T chunk).
- Tiles are sorted per head on the host; (head, key-block, query-chunk) pairs
  that are fully masked on ALL cores are skipped at compile time, pairs that
  need no mask on ANY core skip the mask ops. The causal mask is a 0/1
  multiply on DVE (was: tensor_scalar on GpSimd — the old bottleneck).
- Softmax denominators come from a DVE pre-add of the prob tiles plus a single
  ones-matmul per (qc, head).
- Output partials are written in bf16 and summed on the host in f32.

Self-contained: hardcodes all shapes from the problem spec.
"""

import numpy as np
from contextlib import ExitStack

import concourse.bass as bass
import concourse.tile as tile
from concourse import bacc, mybir
from concourse import bass_utils

# Problem constants
B, S, DM = 2, 4096, 2048
H, D = 16, 128
TILE, NSEL = 16, 64
K = NSEL * TILE  # 1024 selected keys per head

# Per-core constants
NH = 4           # heads per core
P = 128
DMC = DM // P    # 16 contraction chunks
KB = K // P      # 8 key blocks per head
QC = S // 512    # 8 query 512-chunks
TOKC = S // 512  # 8 token 512-chunks (phase A)

F32 = mybir.dt.float32
BF16 = mybir.dt.bfloat16
I16 = mybir.dt.int16


# ---------------------------------------------------------------------------
# classification: which (qc, h, b) logits blocks exist / need masking
# ---------------------------------------------------------------------------

def _sorted_tokens(anchor):
    """tok[core, h_local, 1024] sorted ascending, with the forced last tile."""
    anchor = np.asarray(anchor)
    tok = np.empty((8, NH, K), dtype=np.int64)
    for core in range(8):
        b, hg = core // 4, core % 4
        for h in range(NH):
            tiles = anchor[b, 4 * hg + h].astype(np.int64).copy()
            tiles[-1] = (S - 1) // TILE
            tiles = np.sort(tiles)
            tok[core, h] = (tiles[:, None] * TILE + np.arange(TILE)).reshape(-1)
    return tok


def classify(anchor):
    """Union classification across the 8 cores sharing one NEFF.

    Returns (kept, partial_order, fixqc):
      kept[(qc, h)] = tuple of key-blocks b to compute
      partial_order = tuple of (qc, h, b) triples needing a mask, in the
        canonical order that also indexes the mt table columns
      fixqc = tuple of query chunks that may contain all-masked query rows
    """
    tok = _sorted_tokens(anchor)
    mn = tok[:, :, ::P].min(axis=0)            # [NH, KB] min over cores of block-min
    mx = tok[:, :, P - 1::P].max(axis=0)       # [NH, KB] max over cores of block-max
    kept = {}
    partial_order = []
    for qc in range(QC):
        for h in range(NH):
            bl = []
            for b in range(KB):
                if mn[h, b] > qc * 512 + 511:
                    continue                    # fully masked on every core
                bl.append(b)
                if mx[h, b] > qc * 512:
                    partial_order.append((qc, h, b))
            kept[(qc, h)] = tuple(bl)
    maxtok0 = int(tok[:, :, 0].max())
    fixqc = tuple(qc for qc in range(QC) if qc * 512 < maxtok0)
    return kept, tuple(partial_order), fixqc


# ---------------------------------------------------------------------------
# kernel build
# ---------------------------------------------------------------------------

def build_nc(cls, dbg=False):
    kept, partial_order, fixqc = cls
    npart = max(1, len(partial_order))

    nc = bacc.Bacc("TRN2", target_bir_lowering=False, debug=False, num_devices=8)

    xT_d = nc.dram_tensor("xT", [DM, S], BF16, kind="ExternalInput").ap()
    xg_d = nc.dram_tensor("xg", [S, DM], BF16, kind="ExternalInput").ap()
    # weights are host-prepped in SBUF layout [128, DMC*NH*D] (16KB rows)
    wq_d = nc.dram_tensor("wq", [P, DMC * NH * D], BF16, kind="ExternalInput").ap()
    wk_d = nc.dram_tensor("wk", [P, DMC * NH * D], BF16, kind="ExternalInput").ap()
    wv_d = nc.dram_tensor("wv", [P, DMC * NH * D], BF16, kind="ExternalInput").ap()
    wo_d = nc.dram_tensor("wo", [P, NH * DM], BF16, kind="ExternalInput").ap()
    gidx_d = nc.dram_tensor("gidx", [P, NH * (K // 16)], I16, kind="ExternalInput").ap()
    mt_d = nc.dram_tensor("mt", [P, npart], F32, kind="ExternalInput").ap()
    out_d = nc.dram_tensor("out", [S, DM], BF16, kind="ExternalOutput").ap()
    dbg_d = (nc.dram_tensor("dbg", [P, 16384], BF16, kind="ExternalOutput").ap()
             if dbg else None)

    # NEFF-embedded constants
    import ml_dtypes
    bf = ml_dtypes.bfloat16
    iota_np = np.broadcast_to(np.arange(512, dtype=np.float32), (P, 512)).copy()
    ones_np = np.ones((P, 1), dtype=bf)
    oinv_np = np.full((P, 1), 1.0 / K, dtype=bf)
    onesr_np = np.ones((1, P), dtype=bf)
    onesrow_np = np.ones((1, 512), dtype=bf)
    iota_d = nc.inline_tensor(iota_np, "iota").ap()
    ones_d = nc.inline_tensor(ones_np, "ones").ap()
    oinv_d = nc.inline_tensor(oinv_np, "oinv").ap()
    onesr_d = nc.inline_tensor(onesr_np, "onesr").ap()
    onesrow_d = nc.inline_tensor(onesrow_np, "onesrow").ap()

    with tile.TileContext(nc) as tc, ExitStack() as ctx:
        emit(ctx, tc, cls,
             xT_d=xT_d, xg_d=xg_d, wq_d=wq_d, wk_d=wk_d, wv_d=wv_d, wo_d=wo_d,
             gidx_d=gidx_d, mt_d=mt_d, out_d=out_d, dbg_d=dbg_d,
             iota_d=iota_d, ones_d=ones_d, oinv_d=oinv_d, onesr_d=onesr_d,
             onesrow_d=onesrow_d)

    nc.compile()
    return nc


def emit(ctx, tc, cls, *, xT_d, xg_d, wq_d, wk_d, wv_d, wo_d, gidx_d, mt_d,
         out_d, dbg_d=None, iota_d, ones_d, oinv_d, onesr_d, onesrow_d):
    kept, partial_order, fixqc = cls
    pcol = {t: i for i, t in enumerate(partial_order)}
    fixqc = set(fixqc)
    nc = tc.nc
    AL = mybir.AluOpType
    AF = mybir.ActivationFunctionType

    # ---------------- persistent tiles ----------------
    cpool = ctx.enter_context(tc.tile_pool(name="const", bufs=1))
    iota = cpool.tile([P, 512], F32, tag="iota")
    ones = cpool.tile([P, 1], BF16, tag="ones")
    oinv = cpool.tile([P, 1], BF16, tag="oinv")
    onesr = cpool.tile([1, P], BF16, tag="onesr")
    onesrow = cpool.tile([1, 512], BF16, tag="onesrow")
    gidx = cpool.tile([P, NH * (K // 16)], I16, tag="gidx")
    mt = cpool.tile([P, max(1, len(partial_order))], F32, tag="mt")
    nc.sync.dma_start(iota[:], iota_d[:, :])
    nc.sync.dma_start(ones[:], ones_d[:, :])
    nc.sync.dma_start(oinv[:], oinv_d[:, :])
    nc.sync.dma_start(onesr[:], onesr_d[:, :])
    nc.sync.dma_start(onesrow[:], onesrow_d[:, :])
    nc.sync.dma_start(gidx[:], gidx_d[:, :])
    nc.sync.dma_start(mt[:], mt_d[:, :])

    qpool = ctx.enter_context(tc.tile_pool(name="qT", bufs=1))
    qT = [qpool.tile([P, S], BF16, tag=f"qT{h}", name=f"qT{h}") for h in range(NH)]

    kvpool = ctx.enter_context(tc.tile_pool(name="kv", bufs=1))
    kT = [kvpool.tile([P, K], BF16, tag=f"kT{h}", name=f"kT{h}") for h in range(NH)]
    vsb = [kvpool.tile([P, K], BF16, tag=f"v{h}", name=f"v{h}") for h in range(NH)]
    vsum = [kvpool.tile([1, D], BF16, tag=f"vsum{h}", name=f"vsum{h}")
            for h in range(NH)]

    # ---------------- phases A+B interleaved ----------------
    # A: qT[h] [d=128, tok] = sum_c wq[c,h].T @ xT[c, tok], 8 token chunks.
    # B: per (h, half): K into kT layout directly, V into vsb layout, fed by
    #    dma_gather(transpose=True) tiles. B half i is emitted after A chunk
    #    i+1 so gathers have 2 chunks of PE time to land.
    halves = [(h, hf) for h in range(NH) for hf in range(2)]
    bstack = ExitStack()  # closed after phase B to free the gather buffers
    xtsp = bstack.enter_context(tc.tile_pool(name="xts", bufs=3))
    gath = {}

    def issue_gather(i):
        h, hf = halves[i]
        t = xtsp.tile([P, DMC, 512], BF16, tag="xts", name=f"xts{h}_{hf}")
        col = h * 64 + hf * 32
        nc.gpsimd.dma_gather(
            t[:], xg_d[:, :], gidx[:, col:col + 32], 512, 512, DM,
            transpose=True)
        gath[(h, hf)] = t

    abstack = ExitStack()
    wqp = abstack.enter_context(tc.tile_pool(name="wqp", bufs=1))
    wkp = abstack.enter_context(tc.tile_pool(name="wkp", bufs=1))
    wvp = abstack.enter_context(tc.tile_pool(name="wvp", bufs=1))
    xA = abstack.enter_context(tc.tile_pool(name="xA", bufs=24))
    psA = abstack.enter_context(tc.tile_pool(name="psA", bufs=2, space="PSUM"))
    psK = abstack.enter_context(tc.tile_pool(name="psK", bufs=2, space="PSUM"))
    psV = abstack.enter_context(tc.tile_pool(name="psV", bufs=2, space="PSUM"))
    psVS = abstack.enter_context(tc.tile_pool(name="psVS", bufs=1, space="PSUM"))

    wq_sb = wqp.tile([P, DMC * NH * D], BF16, tag="wq")
    wk_sb = wkp.tile([P, DMC * NH * D], BF16, tag="wk")
    wv_sb = wvp.tile([P, DMC * NH * D], BF16, tag="wv")
    # startup DMA priority: wq, first x pair, first gathers, wk/wv
    nc.sync.dma_start(wq_sb[:], wq_d[:, :])
    xA_tiles = {}

    def emit_xa_dmas(tp):
        # one [128, 1024] DMA per contraction chunk covering token chunks
        # 2*tp and 2*tp+1 (2KB contiguous per partition line)
        tiles = []
        for c in range(DMC):
            xt = xA.tile([P, 1024], BF16, tag="xA")
            nc.sync.dma_start(
                xt[:], xT_d[c * P:(c + 1) * P, tp * 1024:(tp + 1) * 1024])
            tiles.append(xt)
        xA_tiles[tp] = tiles

    emit_xa_dmas(0)
    issue_gather(0)
    issue_gather(1)
    nc.sync.dma_start(wk_sb[:], wk_d[:, :])
    nc.sync.dma_start(wv_sb[:], wv_d[:, :])

    def emit_a_chunk(t):
        xts = xA_tiles[t // 2]
        lo = (t % 2) * 512
        for h in range(NH):
            ps = psA.tile([P, 512], F32)
            for c in range(DMC):
                nc.tensor.matmul(
                    ps[:],
                    lhsT=wq_sb[:, c * 512 + h * P: c * 512 + (h + 1) * P],
                    rhs=xts[c][:, lo:lo + 512],
                    start=(c == 0), stop=(c == DMC - 1))
            nc.vector.tensor_copy(qT[h][:, t * 512:(t + 1) * 512], ps[:])
        if t % 2 == 1:
            del xA_tiles[t // 2]

    def emit_b_half(i):
        h, hf = halves[i]
        xts = gath[(h, hf)]
        # K: kT[h][:, hf*512:(hf+1)*512] = sum_c wk_c.T @ xts_c
        psk = psK.tile([P, 512], F32)
        for c in range(DMC):
            nc.tensor.matmul(
                psk[:],
                lhsT=wk_sb[:, c * 512 + h * P: c * 512 + (h + 1) * P],
                rhs=xts[:, c, :],
                start=(c == 0), stop=(c == DMC - 1))
        nc.vector.tensor_copy(kT[h][:, hf * 512:(hf + 1) * 512], psk[:])
        # V: vsb[h][:, (hf*4+kbl)*128 + d] = gathered_x @ wv  [tok, d]
        psv = psV.tile([P, 512], F32)
        for kbl in range(4):
            for c in range(DMC):
                nc.tensor.matmul(
                    psv[:, kbl * P:(kbl + 1) * P],
                    lhsT=xts[:, c, kbl * P:(kbl + 1) * P],
                    rhs=wv_sb[:, c * 512 + h * P: c * 512 + (h + 1) * P],
                    start=(c == 0), stop=(c == DMC - 1))
        nc.vector.tensor_copy(vsb[h][:, hf * 512:(hf + 1) * 512], psv[:])
        if hf == 1:
            # vsum[h] = (1/K) * sum over all selected tokens of v
            pvs = psVS.tile([1, D], F32)
            for kb in range(KB):
                nc.tensor.matmul(
                    pvs[:], lhsT=oinv[:], rhs=vsb[h][:, kb * P:(kb + 1) * P],
                    start=(kb == 0), stop=(kb == KB - 1))
            nc.vector.tensor_copy(vsum[h][:], pvs[:])

    for t in range(TOKC):
        emit_a_chunk(t)
        if t % 2 == 0 and t + 2 < TOKC:
            emit_xa_dmas((t + 2) // 2)
        if t >= 1:
            emit_b_half(t - 1)
        if t + 2 < TOKC:
            issue_gather(t + 2)
    emit_b_half(6)
    emit_b_half(7)
    abstack.close()

    if dbg_d is not None:
        nc.sync.dma_start(dbg_d[:, 0:4096], qT[0][:, :])
        nc.sync.dma_start(dbg_d[:, 4096:5120], kT[0][:, :])
        nc.sync.dma_start(dbg_d[:, 5120:6144], vsb[0][:, :])
        nc.sync.dma_start(dbg_d[0:1, 6144:6272], vsum[0][:, :])

    bstack.close()

    # ---------------- phase C: attention + Wo ----------------
    with tc.tile_pool(name="wop", bufs=1) as wop, \
         tc.tile_pool(name="ptp", bufs=22) as ptp, \
         tc.tile_pool(name="indp", bufs=8) as indp, \
         tc.tile_pool(name="attnp", bufs=8) as attnp, \
         tc.tile_pool(name="rowp", bufs=2) as rowp, \
         tc.tile_pool(name="rbcp", bufs=3) as rbcp, \
         tc.tile_pool(name="outp", bufs=4) as outp, \
         tc.tile_pool(name="psL", bufs=3, space="PSUM") as psL, \
         tc.tile_pool(name="psO", bufs=2, space="PSUM") as psO, \
         tc.tile_pool(name="psSum", bufs=1, space="PSUM") as psSum, \
         tc.tile_pool(name="psW", bufs=2, space="PSUM") as psW:
        wo_sb = wop.tile([P, NH * DM], BF16, tag="wo")
        nc.sync.dma_start(wo_sb[:], wo_d[:, :])

        pending_wo = [None]

        def emit_wo(qc, attn_t):
            for tb in range(4):
                osb = outp.tile([P, DM], BF16, tag="osb")
                for n in range(4):
                    pw = psW.tile([P, 512], F32, tag="pw",
                                  name=f"pw{qc}_{tb}_{n}")
                    for hh in range(NH):
                        nc.tensor.matmul(
                            pw[:],
                            lhsT=attn_t[hh][:, tb * P:(tb + 1) * P],
                            rhs=wo_sb[:, hh * DM + n * 512: hh * DM + (n + 1) * 512],
                            start=(hh == 0), stop=(hh == NH - 1))
                    if (tb + n) % 2 == 0:
                        nc.vector.tensor_copy(osb[:, n * 512:(n + 1) * 512], pw[:])
                    else:
                        nc.scalar.copy(osb[:, n * 512:(n + 1) * 512], pw[:])
                nc.sync.dma_start(
                    out_d[qc * 512 + tb * P: qc * 512 + (tb + 1) * P, :],
                    osb[:])

        for qc in range(QC):
            sums = psSum.tile([P, 512], F32, tag="sums", name=f"sums{qc}")
            attn_t = {}
            po_t = {}
            use_t = {}
            padd_t = {}

            def stage1(h):
                """logits + exp + mask + incremental denominator pre-add"""
                bl = kept[(qc, h)]
                uses = []
                padd = None
                for b in bl:
                    pl = psL.tile([P, 512], F32)
                    nc.tensor.matmul(
                        pl[:],
                        lhsT=kT[h][:, b * P:(b + 1) * P],
                        rhs=qT[h][:, qc * 512:(qc + 1) * 512],
                        start=True, stop=True)
                    pt = ptp.tile([P, 512], BF16, tag="pt")
                    nc.scalar.activation(pt[:], pl[:], AF.Exp)
                    if (qc, h, b) in pcol:
                        col = pcol[(qc, h, b)]
                        ind = indp.tile([P, 512], BF16, tag="ind")
                        nc.vector.tensor_scalar(
                            out=ind[:], in0=iota[:], scalar1=mt[:, col:col + 1],
                            scalar2=None, op0=AL.is_ge)
                        ptm = ptp.tile([P, 512], BF16, tag="pt")
                        nc.vector.tensor_tensor(
                            out=ptm[:], in0=pt[:], in1=ind[:], op=AL.mult)
                        uses.append(ptm)
                        if dbg_d is not None and qc == 4 and h == 0 and b == bl[-1]:
                            nc.sync.dma_start(dbg_d[:, 6784:7296], ind[:])
                    else:
                        uses.append(pt)
                    if dbg_d is not None and qc == 4 and h == 0 and b == bl[-1]:
                        nc.sync.dma_start(dbg_d[:, 6272:6784], uses[-1][:])
                    # incremental pre-add for the softmax denominator
                    if len(uses) == 2:
                        padd = ptp.tile([P, 512], BF16, tag="pt")
                        nc.vector.tensor_tensor(
                            out=padd[:], in0=uses[0][:], in1=uses[1][:],
                            op=AL.add)
                    elif len(uses) > 2:
                        nc.vector.tensor_tensor(
                            out=padd[:], in0=padd[:], in1=uses[-1][:],
                            op=AL.add)
                use_t[h] = uses
                padd_t[h] = padd if padd is not None else (
                    uses[0] if uses else None)

            def stage2(h):
                """PV + denominator matmul + fix for head h"""
                bl = kept[(qc, h)]
                uses = use_t[h]
                po = psO.tile([P, 512], F32, tag="po", name=f"po{qc}_{h}")
                po_t[h] = po
                if not bl:
                    nc.tensor.matmul(po[:], lhsT=vsum[h][:], rhs=onesrow[:],
                                     start=True, stop=True)
                    return
                need_fix = qc in fixqc
                for j, b in enumerate(bl):
                    nc.tensor.matmul(
                        po[:],
                        lhsT=vsb[h][:, b * P:(b + 1) * P],
                        rhs=uses[j][:],
                        start=(j == 0),
                        stop=(j == len(bl) - 1 and not need_fix))
                srow = sums[32 * h:32 * h + 1, :]
                nc.tensor.matmul(srow, lhsT=ones[:], rhs=padd_t[h][:],
                                 start=True, stop=True,
                                 tile_position=(0, 32 * h))
                if need_fix:
                    fixf = rowp.tile([1, 512], F32, tag="fixf")
                    nc.vector.tensor_scalar(
                        out=fixf[:], in0=srow, scalar1=0.0, scalar2=None,
                        op0=AL.is_equal)
                    fixb = rowp.tile([1, 512], BF16, tag="fixb")
                    nc.vector.tensor_copy(fixb[:], fixf[:])
                    sumb = rowp.tile([1, 512], F32, tag="sumb")
                    nc.vector.tensor_tensor(
                        out=sumb[:], in0=srow, in1=fixf[:], op=AL.add)
                    nc.tensor.matmul(po[:], lhsT=vsum[h][:], rhs=fixb[:],
                                     start=False, stop=True)
                    sin = sumb[:]
                else:
                    # reciprocal_approx_accurate mis-reads PSUM rows at
                    # partition offset != 0 — stage through a partition-0
                    # SBUF row first.
                    scp = rowp.tile([1, 512], F32, tag="scp")
                    nc.vector.tensor_copy(scp[:], srow)
                    sin = scp[:]
                rsc = rowp.tile([1, 512], F32, tag="rsc")
                rss = rowp.tile([1, 512], F32, tag="rss")
                nc.vector.reciprocal_approx_accurate(
                    out=rsc[:], in_=sin, scratch=rss[:])
                rrow = rowp.tile([1, 512], BF16, tag="rrow")
                nc.vector.tensor_copy(rrow[:], rsc[:])
                if dbg_d is not None and qc == 1:
                    nc.sync.dma_start(dbg_d[0:1, 8320 + 512 * h:8320 + 512 * (h + 1)],
                                      rrow[:])
                use_t[h] = rrow  # stash for stage3

            def stage3(h):
                """broadcast reciprocal + normalize head h"""
                po = po_t[h]
                at = attnp.tile([P, 512], BF16, tag="attn", name=f"at{qc}_{h}")
                if not kept[(qc, h)]:
                    nc.vector.tensor_copy(at[:], po[:])
                    attn_t[h] = at
                    return
                rrow = use_t[h]
                pbt = psW.tile([P, 512], F32, tag="pw", name=f"pbt{qc}_{h}")
                nc.tensor.matmul(pbt[:], lhsT=onesr[:], rhs=rrow[:],
                                 start=True, stop=True)
                rbc = rbcp.tile([P, 512], BF16, tag="rbc")
                nc.scalar.copy(rbc[:], pbt[:])
                nc.vector.tensor_tensor(
                    out=at[:], in0=po[:], in1=rbc[:], op=AL.mult)
                if dbg_d is not None and qc == 4 and h == 0:
                    nc.sync.dma_start(dbg_d[:, 7296:7808], at[:])
                if dbg_d is not None and qc == 1:
                    nc.sync.dma_start(dbg_d[:, 10368 + 512 * h:10368 + 512 * (h + 1)],
                                      at[:])
                attn_t[h] = at

            # 3-stage head pipeline; previous qc's Wo is emitted after this
            # qc's first two logit blocks so its attn inputs have slack.
            stage1(0)
            stage1(1)
            if pending_wo[0] is not None:
                pending_wo[0]()
                pending_wo[0] = None
            for fn, h in [(stage2, 0), (stage1, 2), (stage2, 1), (stage3, 0),
                          (stage1, 3), (stage2, 2), (stage3, 1), (stage2, 3),
                          (stage3, 2), (stage3, 3)]:
                fn(h)

            if dbg_d is not None and qc == 1:
                sdump = outp.tile([P, 512], BF16, tag="osb")
                nc.vector.tensor_copy(sdump[:], sums[:])
                nc.sync.dma_start(dbg_d[:, 12416:12928], sdump[:])

            pending_wo[0] = (lambda qc=qc, attn_t=attn_t: emit_wo(qc, attn_t))

        pending_wo[0]()


# ---------------------------------------------------------------------------
# host side
# ---------------------------------------------------------------------------

def make_in_maps(x, Wq, Wk, Wv, Wo, anchor_indices, cls):
    import ml_dtypes
    bf = ml_dtypes.bfloat16
    kept, partial_order, fixqc = cls
    scale = 1.0 / np.sqrt(np.float32(D))
    x = np.asarray(x, dtype=np.float32)
    Wq = np.asarray(Wq, dtype=np.float32)
    Wk = np.asarray(Wk, dtype=np.float32)
    Wv = np.asarray(Wv, dtype=np.float32)
    Wo = np.asarray(Wo, dtype=np.float32)
    tok = _sorted_tokens(anchor_indices)

    xT_b = [np.ascontiguousarray(x[b].T).astype(bf) for b in range(B)]
    xg_b = [np.ascontiguousarray(x[b]).astype(bf) for b in range(B)]

    in_maps = []
    for core in range(8):
        b, hg = core // 4, core % 4
        sl = slice(4 * hg * D, (4 * hg + 4) * D)

        def sbuf_layout(w):
            # [C*128, N] -> [128, C*N]: row p holds chunk-major slices
            cn = w.shape[0] // P
            return np.ascontiguousarray(
                w.reshape(cn, P, w.shape[1]).transpose(1, 0, 2).reshape(P, -1)
            ).astype(bf)

        wq_c = sbuf_layout(Wq[:, sl] * scale)
        wk_c = sbuf_layout(Wk[:, sl])
        wv_c = sbuf_layout(Wv[:, sl])
        wo_c = sbuf_layout(Wo[sl, :])

        # gather indices: per (h, half) group of 512, entry i wrapped to
        # [i % 16, col + i // 16], replicated across the 8 gpsimd stripes
        gidx_c = np.zeros((16, NH * (K // 16)), dtype=np.int16)
        for h in range(NH):
            for hf in range(2):
                seg = tok[core, h, hf * 512:(hf + 1) * 512].astype(np.int16)
                gidx_c[:, h * 64 + hf * 32: h * 64 + (hf + 1) * 32] = \
                    seg.reshape(32, 16).T
        gidx_c = np.tile(gidx_c, (8, 1))

        npart = max(1, len(partial_order))
        mt_c = np.zeros((P, npart), dtype=np.float32)
        for i, (qc, h, bb) in enumerate(partial_order):
            mt_c[:, i] = tok[core, h, bb * P:(bb + 1) * P] - 512.0 * qc - 0.5

        in_maps.append({
            "xT": xT_b[b], "xg": xg_b[b], "wq": wq_c, "wk": wk_c, "wv": wv_c,
            "wo": wo_c, "gidx": gidx_c, "mt": mt_c,
        })
    return in_maps


_NC_CACHE = {}


def get_nc(cls):
    key = (tuple(sorted(cls[0].items())), cls[1], cls[2])
    if key not in _NC_CACHE:
        _NC_CACHE[key] = build_nc(cls)
    return _NC_CACHE[key]


def _ensure_axon_hook_stub():
    # The NTFF profile hook module is absent in some containers; stub it so
    # run_bass_kernel_spmd(trace=True) degrades to a no-trace run.
    import sys, types
    try:
        from antenv import axon_hooks  # noqa: F401
    except ImportError:
        mod = types.ModuleType("antenv.axon_hooks")
        mod.get_axon_ntff_profile_hook = lambda: None
        sys.modules["antenv.axon_hooks"] = mod
        import antenv
        antenv.axon_hooks = mod


def kernel(x, Wq, Wk, Wv, Wo, anchor_indices, _trace=False, _trace_dir=None):
    cls = classify(anchor_indices)
    in_maps = make_in_maps(x, Wq, Wk, Wv, Wo, anchor_indices, cls)
    nc = get_nc(cls)
    if _trace:
        _ensure_axon_hook_stub()
    res = bass_utils.run_bass_kernel_spmd(
        nc, in_maps, core_ids=list(range(8)), trace=_trace, tmpdir=_trace_dir)
    out = np.zeros((B, S, DM), dtype=np.float32)
    for core in range(8):
        out[core // 4] += res.results[core]["out"].astype(np.float32)
    if _trace:
        kernel.last_exec_time_ns = res.exec_time_ns
        kernel.last_results = res
    return out


# revision 39
# speedup vs baseline: 4.8637x; 1.0347x over previous
"""Kascade reuse attention (sparse tile attention) on 8 TRN2 NeuronCores.

Sharding: data-parallel over batch (2) x tensor-parallel over head groups (4),
one (batch, head-group-of-4) pair per core. Each core computes
partial_out = attn_out(4 heads) @ Wo[rows of those heads]  -> [S, DM]
and the host sums the 4 partials per batch (the "all-reduce after Wo").

Key design points (v2):
- Selected K/V tokens are gathered from DRAM with dma_gather(transpose=True),
  which lands x^T tiles [dm-chunk, token] directly in SBUF — no PE transposes.
- K is projected straight into kT [d, tok] layout (lhsT = Wk chunk); V is
  projected into [tok, d] layout (lhsT = gathered x^T chunk).
- Tiles are sorted per head on the host; (head, key-block, query-chunk) pairs
  that are fully masked on ALL cores are skipped at compile time, pairs that
  need no mask on ANY core skip the mask ops. The causal mask is a 0/1
  multiply on DVE (was: tensor_scalar on GpSimd — the old bottleneck).
- Softmax denominators come from a DVE pre-add of the prob tiles plus a single
  ones-matmul per (qc, head).
- Output partials are written in bf16 and summed on the host in f32.

Self-contained: hardcodes all shapes from the problem spec.
"""

import numpy as np
from contextlib import ExitStack

import concourse.bass as bass
import concourse.tile as tile
from concourse import bacc, mybir
from concourse import bass_utils

# Problem constants
B, S, DM = 2, 4096, 2048
H, D = 16, 128
TILE, NSEL = 16, 64
K = NSEL * TILE  # 1024 selected keys per head

# Per-core constants
NH = 4           # heads per core
P = 128
DMC = DM // P    # 16 contraction chunks
KB = K // P      # 8 key blocks per head
QC = S // 512    # 8 query 512-chunks
TOKC = S // 512  # 8 token 512-chunks (phase A)

F32 = mybir.dt.float32
BF16 = mybir.dt.bfloat16
I16 = mybir.dt.int16


# ---------------------------------------------------------------------------
# classification: which (qc, h, b) logits blocks exist / need masking
# ---------------------------------------------------------------------------

def _sorted_tokens(anchor):
    """tok[core, h_local, 1024] sorted ascending, with the forced last tile."""
    anchor = np.asarray(anchor)
    tok = np.empty((8, NH, K), dtype=np.int64)
    for core in range(8):
        b, hg = core // 4, core % 4
        for h in range(NH):
            tiles = anchor[b, 4 * hg + h].astype(np.int64).copy()
            tiles[-1] = (S - 1) // TILE
            tiles = np.sort(tiles)
            tok[core, h] = (tiles[:, None] * TILE + np.arange(TILE)).reshape(-1)
    return tok


def classify(anchor):
    """Union classification across the 8 cores sharing one NEFF.

    Returns (kept, partial_order, fixqc):
      kept[(qc, h)] = tuple of key-blocks b to compute
      partial_order = tuple of (qc, h, b) triples needing a mask, in the
        canonical order that also indexes the mt table columns
      fixqc = tuple of query chunks that may contain all-masked query rows
    """
    tok = _sorted_tokens(anchor)
    mn = tok[:, :, ::P].min(axis=0)            # [NH, KB] min over cores of block-min
    mx = tok[:, :, P - 1::P].max(axis=0)       # [NH, KB] max over cores of block-max
    kept = {}
    partial_order = []
    for qc in range(QC):
        for h in range(NH):
            bl = []
            for b in range(KB):
                if mn[h, b] > qc * 512 + 511:
                    continue                    # fully masked on every core
                bl.append(b)
                if mx[h, b] > qc * 512:
                    partial_order.append((qc, h, b))
            kept[(qc, h)] = tuple(bl)
    maxtok0 = int(tok[:, :, 0].max())
    fixqc = tuple(qc for qc in range(QC) if qc * 512 < maxtok0)
    return kept, tuple(partial_order), fixqc


# ---------------------------------------------------------------------------
# kernel build
# ---------------------------------------------------------------------------

def build_nc(cls, dbg=False):
    kept, partial_order, fixqc = cls
    npart = max(1, len(partial_order))

    nc = bacc.Bacc("TRN2", target_bir_lowering=False, debug=False, num_devices=8)

    xT_d = nc.dram_tensor("xT", [DM, S], BF16, kind="ExternalInput").ap()
    xg_d = nc.dram_tensor("xg", [S, DM], BF16, kind="ExternalInput").ap()
    # weights are host-prepped in SBUF layout [128, DMC*NH*D] (16KB rows)
    wq_d = nc.dram_tensor("wq", [P, DMC * NH * D], BF16, kind="ExternalInput").ap()
    wk_d = nc.dram_tensor("wk", [P, DMC * NH * D], BF16, kind="ExternalInput").ap()
    wv_d = nc.dram_tensor("wv", [P, DMC * NH * D], BF16, kind="ExternalInput").ap()
    wo_d = nc.dram_tensor("wo", [P, NH * DM], BF16, kind="ExternalInput").ap()
    gidx_d = nc.dram_tensor("gidx", [P, NH * (K // 16)], I16, kind="ExternalInput").ap()
    mt_d = nc.dram_tensor("mt", [P, npart], F32, kind="ExternalInput").ap()
    out_d = nc.dram_tensor("out", [S, DM], BF16, kind="ExternalOutput").ap()
    dbg_d = (nc.dram_tensor("dbg", [P, 16384], BF16, kind="ExternalOutput").ap()
             if dbg else None)

    # NEFF-embedded constants
    import ml_dtypes
    bf = ml_dtypes.bfloat16
    iota_np = np.broadcast_to(np.arange(512, dtype=np.float16), (P, 512)).copy()
    ones_np = np.ones((P, 1), dtype=bf)
    oinv_np = np.full((P, 1), 1.0 / K, dtype=bf)
    onesall_np = np.ones((P, P), dtype=bf)
    onesrow_np = np.ones((1, 512), dtype=bf)
    iota_d = nc.inline_tensor(iota_np, "iota").ap()
    ones_d = nc.inline_tensor(ones_np, "ones").ap()
    oinv_d = nc.inline_tensor(oinv_np, "oinv").ap()
    onesr_d = nc.inline_tensor(onesall_np, "onesall").ap()
    onesrow_d = nc.inline_tensor(onesrow_np, "onesrow").ap()

    with tile.TileContext(nc) as tc, ExitStack() as ctx:
        emit(ctx, tc, cls,
             xT_d=xT_d, xg_d=xg_d, wq_d=wq_d, wk_d=wk_d, wv_d=wv_d, wo_d=wo_d,
             gidx_d=gidx_d, mt_d=mt_d, out_d=out_d, dbg_d=dbg_d,
             iota_d=iota_d, ones_d=ones_d, oinv_d=oinv_d, onesr_d=onesr_d,
             onesrow_d=onesrow_d)

    nc.compile()
    return nc


def emit(ctx, tc, cls, *, xT_d, xg_d, wq_d, wk_d, wv_d, wo_d, gidx_d, mt_d,
         out_d, dbg_d=None, iota_d, ones_d, oinv_d, onesr_d, onesrow_d):
    kept, partial_order, fixqc = cls
    pcol = {t: i for i, t in enumerate(partial_order)}
    fixqc = set(fixqc)
    nc = tc.nc
    AL = mybir.AluOpType
    AF = mybir.ActivationFunctionType

    # ---------------- persistent tiles ----------------
    F16 = mybir.dt.float16
    cpool = ctx.enter_context(tc.tile_pool(name="const", bufs=1))
    iota = cpool.tile([P, 512], F16, tag="iota")
    ones = cpool.tile([P, 1], BF16, tag="ones")
    oinv = cpool.tile([P, 1], BF16, tag="oinv")
    onesall = cpool.tile([P, P], BF16, tag="onesall")
    onesrow = cpool.tile([1, 512], BF16, tag="onesrow")
    gidx = cpool.tile([P, NH * (K // 16)], I16, tag="gidx")
    mt = cpool.tile([P, max(1, len(partial_order))], F32, tag="mt")
    nc.sync.dma_start(iota[:], iota_d[:, :])
    nc.sync.dma_start(ones[:], ones_d[:, :])
    nc.sync.dma_start(oinv[:], oinv_d[:, :])
    nc.sync.dma_start(onesall[:], onesr_d[:, :])
    nc.sync.dma_start(onesrow[:], onesrow_d[:, :])
    nc.sync.dma_start(gidx[:], gidx_d[:, :])
    nc.sync.dma_start(mt[:], mt_d[:, :])

    qpool = ctx.enter_context(tc.tile_pool(name="qT", bufs=1))
    qT = [qpool.tile([P, S], BF16, tag=f"qT{h}", name=f"qT{h}") for h in range(NH)]

    kvpool = ctx.enter_context(tc.tile_pool(name="kv", bufs=1))
    kT = [kvpool.tile([P, K], BF16, tag=f"kT{h}", name=f"kT{h}") for h in range(NH)]
    vsb = [kvpool.tile([P, K], BF16, tag=f"v{h}", name=f"v{h}") for h in range(NH)]
    vsum = [kvpool.tile([1, D], BF16, tag=f"vsum{h}", name=f"vsum{h}")
            for h in range(NH)]

    # ---------------- phases A+B interleaved ----------------
    # A: qT[h] [d=128, tok] = sum_c wq[c,h].T @ xT[c, tok], 8 token chunks.
    # B: per (h, half): K into kT layout directly, V into vsb layout, fed by
    #    dma_gather(transpose=True) tiles. B half i is emitted after A chunk
    #    i+1 so gathers have 2 chunks of PE time to land.
    halves = [(h, hf) for h in range(NH) for hf in range(2)]
    bstack = ExitStack()  # closed after phase B to free the gather buffers
    xtsp = bstack.enter_context(tc.tile_pool(name="xts", bufs=3))
    gath = {}

    def issue_gather(i):
        h, hf = halves[i]
        t = xtsp.tile([P, DMC, 512], BF16, tag="xts", name=f"xts{h}_{hf}")
        col = h * 64 + hf * 32
        nc.gpsimd.dma_gather(
            t[:], xg_d[:, :], gidx[:, col:col + 32], 512, 512, DM,
            transpose=True)
        gath[(h, hf)] = t

    abstack = ExitStack()
    wqp = abstack.enter_context(tc.tile_pool(name="wqp", bufs=1))
    wkp = abstack.enter_context(tc.tile_pool(name="wkp", bufs=1))
    wvp = abstack.enter_context(tc.tile_pool(name="wvp", bufs=1))
    xA = abstack.enter_context(tc.tile_pool(name="xA", bufs=24))
    psA = abstack.enter_context(tc.tile_pool(name="psA", bufs=3, space="PSUM"))
    psK = abstack.enter_context(tc.tile_pool(name="psK", bufs=2, space="PSUM"))
    psV = abstack.enter_context(tc.tile_pool(name="psV", bufs=2, space="PSUM"))
    psVS = abstack.enter_context(tc.tile_pool(name="psVS", bufs=1, space="PSUM"))

    wq_sb = wqp.tile([P, DMC * NH * D], BF16, tag="wq")
    wk_sb = wkp.tile([P, DMC * NH * D], BF16, tag="wk")
    wv_sb = wvp.tile([P, DMC * NH * D], BF16, tag="wv")
    # startup DMA priority: wq (split so early chunks land first), first x
    # pair, first gathers, wk/wv
    for j in range(4):
        nc.sync.dma_start(wq_sb[:, j * 2048:(j + 1) * 2048],
                          wq_d[:, j * 2048:(j + 1) * 2048])
    xA_tiles = {}

    def emit_xa_dmas(tp):
        # one [128, 1024] DMA per contraction chunk covering token chunks
        # 2*tp and 2*tp+1 (2KB contiguous per partition line)
        tiles = []
        for c in range(DMC):
            xt = xA.tile([P, 1024], BF16, tag="xA")
            nc.sync.dma_start(
                xt[:], xT_d[c * P:(c + 1) * P, tp * 1024:(tp + 1) * 1024])
            tiles.append(xt)
        xA_tiles[tp] = tiles

    emit_xa_dmas(0)
    issue_gather(0)
    issue_gather(1)
    for j in range(4):
        nc.sync.dma_start(wk_sb[:, j * 2048:(j + 1) * 2048],
                          wk_d[:, j * 2048:(j + 1) * 2048])
        nc.sync.dma_start(wv_sb[:, j * 2048:(j + 1) * 2048],
                          wv_d[:, j * 2048:(j + 1) * 2048])

    def emit_a_chunk(t):
        xts = xA_tiles[t // 2]
        lo = (t % 2) * 512
        for h in range(NH):
            ps = psA.tile([P, 512], F32)
            for c in range(DMC):
                nc.tensor.matmul(
                    ps[:],
                    lhsT=wq_sb[:, c * 512 + h * P: c * 512 + (h + 1) * P],
                    rhs=xts[c][:, lo:lo + 512],
                    start=(c == 0), stop=(c == DMC - 1))
            nc.vector.tensor_copy(qT[h][:, t * 512:(t + 1) * 512], ps[:])
        if t % 2 == 1:
            del xA_tiles[t // 2]

    def emit_b_half(i):
        h, hf = halves[i]
        xts = gath[(h, hf)]
        # K: kT[h][:, hf*512:(hf+1)*512] = sum_c wk_c.T @ xts_c
        psk = psK.tile([P, 512], F32)
        for c in range(DMC):
            nc.tensor.matmul(
                psk[:],
                lhsT=wk_sb[:, c * 512 + h * P: c * 512 + (h + 1) * P],
                rhs=xts[:, c, :],
                start=(c == 0), stop=(c == DMC - 1))
        nc.vector.tensor_copy(kT[h][:, hf * 512:(hf + 1) * 512], psk[:])
        # V: vsb[h][:, (hf*4+kbl)*128 + d] = gathered_x @ wv  [tok, d]
        psv = psV.tile([P, 512], F32)
        for kbl in range(4):
            for c in range(DMC):
                nc.tensor.matmul(
                    psv[:, kbl * P:(kbl + 1) * P],
                    lhsT=xts[:, c, kbl * P:(kbl + 1) * P],
                    rhs=wv_sb[:, c * 512 + h * P: c * 512 + (h + 1) * P],
                    start=(c == 0), stop=(c == DMC - 1))
        nc.vector.tensor_copy(vsb[h][:, hf * 512:(hf + 1) * 512], psv[:])
        if hf == 1:
            # vsum[h] = (1/K) * sum over all selected tokens of v
            pvs = psVS.tile([1, D], F32)
            for kb in range(KB):
                nc.tensor.matmul(
                    pvs[:], lhsT=oinv[:], rhs=vsb[h][:, kb * P:(kb + 1) * P],
                    start=(kb == 0), stop=(kb == KB - 1))
            nc.vector.tensor_copy(vsum[h][:], pvs[:])

    for t in range(TOKC):
        emit_a_chunk(t)
        if t % 2 == 0 and t + 2 < TOKC:
            emit_xa_dmas((t + 2) // 2)
        if t >= 1:
            emit_b_half(t - 1)
        if t + 2 < TOKC:
            issue_gather(t + 2)
    emit_b_half(6)
    emit_b_half(7)
    abstack.close()

    if dbg_d is not None:
        nc.sync.dma_start(dbg_d[:, 0:4096], qT[0][:, :])
        nc.sync.dma_start(dbg_d[:, 4096:5120], kT[0][:, :])
        nc.sync.dma_start(dbg_d[:, 5120:6144], vsb[0][:, :])
        nc.sync.dma_start(dbg_d[0:1, 6144:6272], vsum[0][:, :])

    bstack.close()

    # ---------------- phase C: attention + Wo ----------------
    with tc.tile_pool(name="wop", bufs=1) as wop, \
         tc.tile_pool(name="ptp", bufs=22) as ptp, \
         tc.tile_pool(name="indp", bufs=8) as indp, \
         tc.tile_pool(name="attnp", bufs=8) as attnp, \
         tc.tile_pool(name="rowp", bufs=2) as rowp, \
         tc.tile_pool(name="sbp", bufs=2) as sbp, \
         tc.tile_pool(name="rbcp", bufs=3) as rbcp, \
         tc.tile_pool(name="outp", bufs=4) as outp, \
         tc.tile_pool(name="psL", bufs=3, space="PSUM") as psL, \
         tc.tile_pool(name="psO", bufs=2, space="PSUM") as psO, \
         tc.tile_pool(name="psSum", bufs=1, space="PSUM") as psSum, \
         tc.tile_pool(name="psW", bufs=2, space="PSUM") as psW:
        wo_sb = wop.tile([P, NH * DM], BF16, tag="wo")
        nc.sync.dma_start(wo_sb[:], wo_d[:, :])

        pending_wo = [None]

        def make_wo_parts(qc, attn_t):
            def part(tb):
                osb = outp.tile([P, DM], BF16, tag="osb")
                for n in range(4):
                    pw = psW.tile([P, 512], F32, tag="pw",
                                  name=f"pw{qc}_{tb}_{n}")
                    for hh in range(NH):
                        nc.tensor.matmul(
                            pw[:],
                            lhsT=attn_t[hh][:, tb * P:(tb + 1) * P],
                            rhs=wo_sb[:, hh * DM + n * 512: hh * DM + (n + 1) * 512],
                            start=(hh == 0), stop=(hh == NH - 1))
                    if (tb + n) % 2 == 0:
                        nc.vector.tensor_copy(osb[:, n * 512:(n + 1) * 512], pw[:])
                    else:
                        nc.scalar.copy(osb[:, n * 512:(n + 1) * 512], pw[:])
                nc.sync.dma_start(
                    out_d[qc * 512 + tb * P: qc * 512 + (tb + 1) * P, :],
                    osb[:])
            return [lambda tb=tb: part(tb) for tb in range(4)]

        def wo_part(i):
            if pending_wo[0] is not None:
                pending_wo[0][i]()

        for qc in range(QC):
            sums = psSum.tile([P, 512], F32, tag="sums", name=f"sums{qc}")
            attn_t = {}
            po_t = {}
            use_t = {}
            padd_t = {}
            rrow_t = {}
            rec_bf = {}

            def stage1(h):
                """logits + exp + mask + incremental denominator pre-add"""
                bl = kept[(qc, h)]
                uses = []
                padd = None
                for b in bl:
                    pl = psL.tile([P, 512], F32)
                    nc.tensor.matmul(
                        pl[:],
                        lhsT=kT[h][:, b * P:(b + 1) * P],
                        rhs=qT[h][:, qc * 512:(qc + 1) * 512],
                        start=True, stop=True)
                    pt = ptp.tile([P, 512], BF16, tag="pt")
                    nc.scalar.activation(pt[:], pl[:], AF.Exp)
                    if (qc, h, b) in pcol:
                        col = pcol[(qc, h, b)]
                        ind = indp.tile([P, 512], BF16, tag="ind")
                        nc.vector.tensor_scalar(
                            out=ind[:], in0=iota[:], scalar1=mt[:, col:col + 1],
                            scalar2=None, op0=AL.is_ge)
                        ptm = ptp.tile([P, 512], BF16, tag="pt")
                        nc.vector.tensor_tensor(
                            out=ptm[:], in0=pt[:], in1=ind[:], op=AL.mult)
                        uses.append(ptm)
                        if dbg_d is not None and qc == 4 and h == 0 and b == bl[-1]:
                            nc.sync.dma_start(dbg_d[:, 6784:7296], ind[:])
                    else:
                        uses.append(pt)
                    if dbg_d is not None and qc == 4 and h == 0 and b == bl[-1]:
                        nc.sync.dma_start(dbg_d[:, 6272:6784], uses[-1][:])
                    # incremental pre-add for the softmax denominator
                    if len(uses) == 2:
                        padd = ptp.tile([P, 512], BF16, tag="pt")
                        nc.vector.tensor_tensor(
                            out=padd[:], in0=uses[0][:], in1=uses[1][:],
                            op=AL.add)
                    elif len(uses) > 2:
                        nc.vector.tensor_tensor(
                            out=padd[:], in0=padd[:], in1=uses[-1][:],
                            op=AL.add)
                use_t[h] = uses
                padd_t[h] = padd if padd is not None else (
                    uses[0] if uses else None)

            def stage2(h):
                """PV + denominator matmul + fix for head h"""
                bl = kept[(qc, h)]
                uses = use_t[h]
                po = psO.tile([P, 512], F32, tag="po", name=f"po{qc}_{h}")
                po_t[h] = po
                if not bl:
                    nc.tensor.matmul(po[:], lhsT=vsum[h][:], rhs=onesrow[:],
                                     start=True, stop=True)
                    return
                need_fix = qc in fixqc
                for j, b in enumerate(bl):
                    nc.tensor.matmul(
                        po[:],
                        lhsT=vsb[h][:, b * P:(b + 1) * P],
                        rhs=uses[j][:],
                        start=(j == 0),
                        stop=(j == len(bl) - 1 and not need_fix))
                srow = sums[32 * h:32 * h + 1, :]
                nc.tensor.matmul(srow, lhsT=ones[:], rhs=padd_t[h][:],
                                 start=True, stop=True,
                                 tile_position=(0, 32 * h))
                if need_fix:
                    fixf = rowp.tile([1, 512], F32, tag="fixf")
                    nc.vector.tensor_scalar(
                        out=fixf[:], in0=srow, scalar1=0.0, scalar2=None,
                        op0=AL.is_equal)
                    fixb = rowp.tile([1, 512], BF16, tag="fixb")
                    nc.vector.tensor_copy(fixb[:], fixf[:])
                    sumb = rowp.tile([1, 512], F32, tag="sumb")
                    nc.vector.tensor_tensor(
                        out=sumb[:], in0=srow, in1=fixf[:], op=AL.add)
                    nc.tensor.matmul(po[:], lhsT=vsum[h][:], rhs=fixb[:],
                                     start=False, stop=True)
                    # per-head reciprocal at partition 0 (fix path only)
                    rsc = rowp.tile([1, 512], F32, tag="rsc")
                    rss = rowp.tile([1, 512], F32, tag="rss")
                    nc.vector.reciprocal_approx_accurate(
                        out=rsc[:], in_=sumb[:], scratch=rss[:])
                    rrow = rowp.tile([1, 512], BF16, tag="rrow")
                    nc.vector.tensor_copy(rrow[:], rsc[:])
                    rrow_t[h] = rrow

            def srecip(half):
                """batched reciprocal for heads 2*half, 2*half+1 (non-fix)"""
                if qc in fixqc:
                    return
                ssb = sbp.tile([64, 512], F32, tag=f"ssb{half}")
                nc.vector.tensor_copy(ssb[:], sums[64 * half:64 * half + 64, :])
                rsc = sbp.tile([64, 512], F32, tag=f"rsc{half}")
                rss = sbp.tile([64, 512], F32, tag=f"rss{half}")
                nc.vector.reciprocal_approx_accurate(
                    out=rsc[:], in_=ssb[:], scratch=rss[:])
                rb = sbp.tile([64, 512], BF16, tag=f"rbf{half}")
                nc.vector.tensor_copy(rb[:], rsc[:])
                rec_bf[half] = rb

            def stage3(h):
                """broadcast reciprocal + normalize head h"""
                po = po_t[h]
                at = attnp.tile([P, 512], BF16, tag="attn", name=f"at{qc}_{h}")
                if not kept[(qc, h)]:
                    nc.vector.tensor_copy(at[:], po[:])
                    attn_t[h] = at
                    return
                pbt = psW.tile([P, 512], F32, tag="pw", name=f"pbt{qc}_{h}")
                if qc in fixqc:
                    nc.tensor.matmul(pbt[:], lhsT=onesall[0:1, :],
                                     rhs=rrow_t[h][:], start=True, stop=True)
                else:
                    ro = 32 * (h % 2)
                    nc.tensor.matmul(
                        pbt[:], lhsT=onesall[ro:ro + 1, :],
                        rhs=rec_bf[h // 2][ro:ro + 1, :],
                        start=True, stop=True, tile_position=(ro, 0))
                rbc = rbcp.tile([P, 512], BF16, tag="rbc")
                nc.vector.tensor_copy(rbc[:], pbt[:])
                nc.vector.tensor_tensor(
                    out=at[:], in0=po[:], in1=rbc[:], op=AL.mult)
                if dbg_d is not None and qc == 4 and h == 0:
                    nc.sync.dma_start(dbg_d[:, 7296:7808], at[:])
                if dbg_d is not None and qc == 1:
                    nc.sync.dma_start(dbg_d[:, 10368 + 512 * h:10368 + 512 * (h + 1)],
                                      at[:])
                attn_t[h] = at

            # pipeline: previous qc's Wo tb-blocks are spread through this
            # qc's early stages; reciprocal batches release po banks in pairs.
            stage1(0)
            wo_part(0)
            stage1(1)
            wo_part(1)
            stage2(0)
            wo_part(2)
            stage1(2)
            stage2(1)
            wo_part(3)
            srecip(0)
            stage3(0)
            stage1(3)
            stage3(1)
            stage2(2)
            stage2(3)
            srecip(1)
            stage3(2)
            stage3(3)

            if dbg_d is not None and qc == 1:
                sdump = outp.tile([P, DM], BF16, tag="osb")
                nc.vector.tensor_copy(sdump[:, 0:512], sums[:])
                nc.sync.dma_start(dbg_d[:, 12416:12928], sdump[:, 0:512])

            pending_wo[0] = make_wo_parts(qc, attn_t)

        for i in range(4):
            wo_part(i)


# ---------------------------------------------------------------------------
# host side
# ---------------------------------------------------------------------------

def make_in_maps(x, Wq, Wk, Wv, Wo, anchor_indices, cls):
    import ml_dtypes
    bf = ml_dtypes.bfloat16
    kept, partial_order, fixqc = cls
    scale = 1.0 / np.sqrt(np.float32(D))
    x = np.asarray(x, dtype=np.float32)
    Wq = np.asarray(Wq, dtype=np.float32)
    Wk = np.asarray(Wk, dtype=np.float32)
    Wv = np.asarray(Wv, dtype=np.float32)
    Wo = np.asarray(Wo, dtype=np.float32)
    tok = _sorted_tokens(anchor_indices)

    xT_b = [np.ascontiguousarray(x[b].T).astype(bf) for b in range(B)]
    xg_b = [np.ascontiguousarray(x[b]).astype(bf) for b in range(B)]

    in_maps = []
    for core in range(8):
        b, hg = core // 4, core % 4
        sl = slice(4 * hg * D, (4 * hg + 4) * D)

        def sbuf_layout(w):
            # [C*128, N] -> [128, C*N]: row p holds chunk-major slices
            cn = w.shape[0] // P
            return np.ascontiguousarray(
                w.reshape(cn, P, w.shape[1]).transpose(1, 0, 2).reshape(P, -1)
            ).astype(bf)

        wq_c = sbuf_layout(Wq[:, sl] * scale)
        wk_c = sbuf_layout(Wk[:, sl])
        wv_c = sbuf_layout(Wv[:, sl])
        wo_c = sbuf_layout(Wo[sl, :])

        # gather indices: per (h, half) group of 512, entry i wrapped to
        # [i % 16, col + i // 16], replicated across the 8 gpsimd stripes
        gidx_c = np.zeros((16, NH * (K // 16)), dtype=np.int16)
        for h in range(NH):
            for hf in range(2):
                seg = tok[core, h, hf * 512:(hf + 1) * 512].astype(np.int16)
                gidx_c[:, h * 64 + hf * 32: h * 64 + (hf + 1) * 32] = \
                    seg.reshape(32, 16).T
        gidx_c = np.tile(gidx_c, (8, 1))

        npart = max(1, len(partial_order))
        mt_c = np.zeros((P, npart), dtype=np.float32)
        for i, (qc, h, bb) in enumerate(partial_order):
            mt_c[:, i] = tok[core, h, bb * P:(bb + 1) * P] - 512.0 * qc - 0.5

        in_maps.append({
            "xT": xT_b[b], "xg": xg_b[b], "wq": wq_c, "wk": wk_c, "wv": wv_c,
            "wo": wo_c, "gidx": gidx_c, "mt": mt_c,
        })
    return in_maps


_NC_CACHE = {}


def get_nc(cls):
    key = (tuple(sorted(cls[0].items())), cls[1], cls[2])
    if key not in _NC_CACHE:
        _NC_CACHE[key] = build_nc(cls)
    return _NC_CACHE[key]


def _ensure_axon_hook_stub():
    # The NTFF profile hook module is absent in some containers; stub it so
    # run_bass_kernel_spmd(trace=True) degrades to a no-trace run.
    import sys, types
    try:
        from antenv import axon_hooks  # noqa: F401
    except ImportError:
        mod = types.ModuleType("antenv.axon_hooks")
        mod.get_axon_ntff_profile_hook = lambda: None
        sys.modules["antenv.axon_hooks"] = mod
        import antenv
        antenv.axon_hooks = mod


def kernel(x, Wq, Wk, Wv, Wo, anchor_indices, _trace=False, _trace_dir=None):
    cls = classify(anchor_indices)
    in_maps = make_in_maps(x, Wq, Wk, Wv, Wo, anchor_indices, cls)
    nc = get_nc(cls)
    if _trace:
        _ensure_axon_hook_stub()
    res = bass_utils.run_bass_kernel_spmd(
        nc, in_maps, core_ids=list(range(8)), trace=_trace, tmpdir=_trace_dir)
    out = np.zeros((B, S, DM), dtype=np.float32)
    for core in range(8):
        out[core // 4] += res.results[core]["out"].astype(np.float32)
    if _trace:
        kernel.last_exec_time_ns = res.exec_time_ns
        kernel.last_results = res
    return out


# revision 52
# speedup vs baseline: 4.8711x; 1.0015x over previous
"""Kascade reuse attention (sparse tile attention) on 8 TRN2 NeuronCores.

Sharding: data-parallel over batch (2) x tensor-parallel over head groups (4),
one (batch, head-group-of-4) pair per core. Each core computes
partial_out = attn_out(4 heads) @ Wo[rows of those heads]  -> [S, DM]
and the host sums the 4 partials per batch (the "all-reduce after Wo").

Key design points (v2):
- Selected K/V tokens are gathered from DRAM with dma_gather(transpose=True),
  which lands x^T tiles [dm-chunk, token] directly in SBUF — no PE transposes.
- K is projected straight into kT [d, tok] layout (lhsT = Wk chunk); V is
  projected into [tok, d] layout (lhsT = gathered x^T chunk).
- Tiles are sorted per head on the host; (head, key-block, query-chunk) pairs
  that are fully masked on ALL cores are skipped at compile time, pairs that
  need no mask on ANY core skip the mask ops. The causal mask is a 0/1
  multiply on DVE (was: tensor_scalar on GpSimd — the old bottleneck).
- Softmax denominators come from a DVE pre-add of the prob tiles plus a single
  ones-matmul per (qc, head).
- Output partials are written in bf16 and summed on the host in f32.

Self-contained: hardcodes all shapes from the problem spec.
"""

import numpy as np
from contextlib import ExitStack

import concourse.bass as bass
import concourse.tile as tile
from concourse import bacc, mybir
from concourse import bass_utils

# Problem constants
B, S, DM = 2, 4096, 2048
H, D = 16, 128
TILE, NSEL = 16, 64
K = NSEL * TILE  # 1024 selected keys per head

# Per-core constants
NH = 4           # heads per core
P = 128
DMC = DM // P    # 16 contraction chunks
KB = K // P      # 8 key blocks per head
QC = S // 512    # 8 query 512-chunks
TOKC = S // 512  # 8 token 512-chunks (phase A)

F32 = mybir.dt.float32
BF16 = mybir.dt.bfloat16
I16 = mybir.dt.int16


# ---------------------------------------------------------------------------
# classification: which (qc, h, b) logits blocks exist / need masking
# ---------------------------------------------------------------------------

def _sorted_tokens(anchor):
    """tok[core, h_local, 1024] sorted ascending, with the forced last tile."""
    anchor = np.asarray(anchor)
    tok = np.empty((8, NH, K), dtype=np.int64)
    for core in range(8):
        b, hg = core // 4, core % 4
        for h in range(NH):
            tiles = anchor[b, 4 * hg + h].astype(np.int64).copy()
            tiles[-1] = (S - 1) // TILE
            tiles = np.sort(tiles)
            tok[core, h] = (tiles[:, None] * TILE + np.arange(TILE)).reshape(-1)
    return tok


def classify(anchor):
    """Union classification across the 8 cores sharing one NEFF.

    Returns (kept, partial_order, fixqc):
      kept[(qc, h)] = tuple of key-blocks b to compute
      partial_order = tuple of (qc, h, b) triples needing a mask, in the
        canonical order that also indexes the mt table columns
      fixqc = tuple of query chunks that may contain all-masked query rows
    """
    tok = _sorted_tokens(anchor)
    mn = tok[:, :, ::P].min(axis=0)            # [NH, KB] min over cores of block-min
    mx = tok[:, :, P - 1::P].max(axis=0)       # [NH, KB] max over cores of block-max
    kept = {}
    partial_order = []
    for qc in range(QC):
        for h in range(NH):
            bl = []
            for b in range(KB):
                if mn[h, b] > qc * 512 + 511:
                    continue                    # fully masked on every core
                bl.append(b)
                if mx[h, b] > qc * 512:
                    partial_order.append((qc, h, b))
            kept[(qc, h)] = tuple(bl)
    maxtok0 = int(tok[:, :, 0].max())
    fixqc = tuple(qc for qc in range(QC) if qc * 512 < maxtok0)
    return kept, tuple(partial_order), fixqc


# ---------------------------------------------------------------------------
# kernel build
# ---------------------------------------------------------------------------

def build_nc(cls, dbg=False):
    kept, partial_order, fixqc = cls
    npart = max(1, len(partial_order))

    nc = bacc.Bacc("TRN2", target_bir_lowering=False, debug=False, num_devices=8)

    xT_d = nc.dram_tensor("xT", [DM, S], BF16, kind="ExternalInput").ap()
    xg_d = nc.dram_tensor("xg", [S, DM], BF16, kind="ExternalInput").ap()
    # weights are host-prepped in SBUF layout [128, DMC*NH*D] (16KB rows)
    wq_d = nc.dram_tensor("wq", [P, DMC * NH * D], BF16, kind="ExternalInput").ap()
    wk_d = nc.dram_tensor("wk", [P, DMC * NH * D], BF16, kind="ExternalInput").ap()
    wv_d = nc.dram_tensor("wv", [P, DMC * NH * D], BF16, kind="ExternalInput").ap()
    wo_d = nc.dram_tensor("wo", [P, NH * DM], BF16, kind="ExternalInput").ap()
    gidx_d = nc.dram_tensor("gidx", [P, NH * (K // 16)], I16, kind="ExternalInput").ap()
    mt_d = nc.dram_tensor("mt", [P, npart], F32, kind="ExternalInput").ap()
    out_d = nc.dram_tensor("out", [S, DM], BF16, kind="ExternalOutput").ap()
    dbg_d = (nc.dram_tensor("dbg", [P, 16384], BF16, kind="ExternalOutput").ap()
             if dbg else None)

    # NEFF-embedded constants
    import ml_dtypes
    bf = ml_dtypes.bfloat16
    iota_np = np.broadcast_to(np.arange(512, dtype=np.float16), (P, 512)).copy()
    ones_np = np.ones((P, 1), dtype=bf)
    oinv_np = np.full((P, 1), 1.0 / K, dtype=bf)
    onesall_np = np.ones((P, P), dtype=bf)
    onesrow_np = np.ones((1, 512), dtype=bf)
    iota_d = nc.inline_tensor(iota_np, "iota").ap()
    ones_d = nc.inline_tensor(ones_np, "ones").ap()
    oinv_d = nc.inline_tensor(oinv_np, "oinv").ap()
    onesr_d = nc.inline_tensor(onesall_np, "onesall").ap()
    onesrow_d = nc.inline_tensor(onesrow_np, "onesrow").ap()

    with tile.TileContext(nc) as tc, ExitStack() as ctx:
        emit(ctx, tc, cls,
             xT_d=xT_d, xg_d=xg_d, wq_d=wq_d, wk_d=wk_d, wv_d=wv_d, wo_d=wo_d,
             gidx_d=gidx_d, mt_d=mt_d, out_d=out_d, dbg_d=dbg_d,
             iota_d=iota_d, ones_d=ones_d, oinv_d=oinv_d, onesr_d=onesr_d,
             onesrow_d=onesrow_d)

    nc.compile()
    return nc


def emit(ctx, tc, cls, *, xT_d, xg_d, wq_d, wk_d, wv_d, wo_d, gidx_d, mt_d,
         out_d, dbg_d=None, iota_d, ones_d, oinv_d, onesr_d, onesrow_d):
    kept, partial_order, fixqc = cls
    pcol = {t: i for i, t in enumerate(partial_order)}
    fixqc = set(fixqc)
    nc = tc.nc
    AL = mybir.AluOpType
    AF = mybir.ActivationFunctionType

    # ---------------- persistent tiles ----------------
    F16 = mybir.dt.float16
    cpool = ctx.enter_context(tc.tile_pool(name="const", bufs=1))
    iota = cpool.tile([P, 512], F16, tag="iota")
    ones = cpool.tile([P, 1], BF16, tag="ones")
    oinv = cpool.tile([P, 1], BF16, tag="oinv")
    onesall = cpool.tile([P, P], BF16, tag="onesall")
    onesrow = cpool.tile([1, 512], BF16, tag="onesrow")
    gidx = cpool.tile([P, NH * (K // 16)], I16, tag="gidx")
    mt = cpool.tile([P, max(1, len(partial_order))], F32, tag="mt")
    nc.sync.dma_start(iota[:], iota_d[:, :])
    nc.sync.dma_start(ones[:], ones_d[:, :])
    nc.sync.dma_start(oinv[:], oinv_d[:, :])
    nc.sync.dma_start(onesall[:], onesr_d[:, :])
    nc.sync.dma_start(onesrow[:], onesrow_d[:, :])
    # gidx/mt are DMA'd later (phase A startup priority)

    qpool = ctx.enter_context(tc.tile_pool(name="qT", bufs=1))
    qT = [qpool.tile([P, S], BF16, tag=f"qT{h}", name=f"qT{h}") for h in range(NH)]

    kvpool = ctx.enter_context(tc.tile_pool(name="kv", bufs=1))
    kT = [kvpool.tile([P, K], BF16, tag=f"kT{h}", name=f"kT{h}") for h in range(NH)]
    vsb = [kvpool.tile([P, K], BF16, tag=f"v{h}", name=f"v{h}") for h in range(NH)]
    vsum = [kvpool.tile([1, D], BF16, tag=f"vsum{h}", name=f"vsum{h}")
            for h in range(NH)]

    # ---------------- phases A+B interleaved ----------------
    # A: qT[h] [d=128, tok] = sum_c wq[c,h].T @ xT[c, tok], 8 token chunks.
    # B: per (h, half): K into kT layout directly, V into vsb layout, fed by
    #    dma_gather(transpose=True) tiles. B half i is emitted after A chunk
    #    i+1 so gathers have 2 chunks of PE time to land.
    halves = [(h, hf) for h in range(NH) for hf in range(2)]
    bstack = ExitStack()  # closed after phase B to free the gather buffers
    xtsp = bstack.enter_context(tc.tile_pool(name="xts", bufs=3))
    gath = {}

    def issue_gather(i):
        h, hf = halves[i]
        t = xtsp.tile([P, DMC, 512], BF16, tag="xts", name=f"xts{h}_{hf}")
        col = h * 64 + hf * 32
        nc.gpsimd.dma_gather(
            t[:], xg_d[:, :], gidx[:, col:col + 32], 512, 512, DM,
            transpose=True)
        gath[(h, hf)] = t

    abstack = ExitStack()
    wqp = abstack.enter_context(tc.tile_pool(name="wqp", bufs=1))
    wkp = abstack.enter_context(tc.tile_pool(name="wkp", bufs=1))
    wvp = abstack.enter_context(tc.tile_pool(name="wvp", bufs=1))
    xA = abstack.enter_context(tc.tile_pool(name="xA", bufs=24))
    psA = abstack.enter_context(tc.tile_pool(name="psA", bufs=3, space="PSUM"))
    psK = abstack.enter_context(tc.tile_pool(name="psK", bufs=2, space="PSUM"))
    psV = abstack.enter_context(tc.tile_pool(name="psV", bufs=2, space="PSUM"))
    psVS = abstack.enter_context(tc.tile_pool(name="psVS", bufs=1, space="PSUM"))

    wq_sb = wqp.tile([P, DMC * NH * D], BF16, tag="wq")
    wk_sb = wkp.tile([P, DMC * NH * D], BF16, tag="wk")
    wv_sb = wvp.tile([P, DMC * NH * D], BF16, tag="wv")
    # startup DMA priority: wq (split so early chunks land first), first x
    # pair, first gathers, wk/wv
    for j in range(4):
        nc.sync.dma_start(wq_sb[:, j * 2048:(j + 1) * 2048],
                          wq_d[:, j * 2048:(j + 1) * 2048])
    xA_tiles = {}

    def emit_xa_dmas(tp):
        # one [128, 1024] DMA per contraction chunk covering token chunks
        # 2*tp and 2*tp+1 (2KB contiguous per partition line)
        tiles = []
        for c in range(DMC):
            xt = xA.tile([P, 1024], BF16, tag="xA")
            nc.sync.dma_start(
                xt[:], xT_d[c * P:(c + 1) * P, tp * 1024:(tp + 1) * 1024])
            tiles.append(xt)
        xA_tiles[tp] = tiles

    emit_xa_dmas(0)
    # gathers wait on gidx — loading it after the first x pair gives phase A
    # startup priority on HBM
    nc.sync.dma_start(gidx[:], gidx_d[:, :])
    nc.sync.dma_start(mt[:], mt_d[:, :])
    issue_gather(0)
    issue_gather(1)
    for j in range(4):
        nc.sync.dma_start(wk_sb[:, j * 2048:(j + 1) * 2048],
                          wk_d[:, j * 2048:(j + 1) * 2048])
        nc.sync.dma_start(wv_sb[:, j * 2048:(j + 1) * 2048],
                          wv_d[:, j * 2048:(j + 1) * 2048])

    def emit_a_chunk(t):
        xts = xA_tiles[t // 2]
        lo = (t % 2) * 512
        for h in range(NH):
            ps = psA.tile([P, 512], F32)
            for c in range(DMC):
                nc.tensor.matmul(
                    ps[:],
                    lhsT=wq_sb[:, c * 512 + h * P: c * 512 + (h + 1) * P],
                    rhs=xts[c][:, lo:lo + 512],
                    start=(c == 0), stop=(c == DMC - 1))
            nc.vector.tensor_copy(qT[h][:, t * 512:(t + 1) * 512], ps[:])
        if t % 2 == 1:
            del xA_tiles[t // 2]

    def emit_b_half(i):
        h, hf = halves[i]
        xts = gath[(h, hf)]
        # K: kT[h][:, hf*512:(hf+1)*512] = sum_c wk_c.T @ xts_c
        psk = psK.tile([P, 512], F32)
        for c in range(DMC):
            nc.tensor.matmul(
                psk[:],
                lhsT=wk_sb[:, c * 512 + h * P: c * 512 + (h + 1) * P],
                rhs=xts[:, c, :],
                start=(c == 0), stop=(c == DMC - 1))
        nc.vector.tensor_copy(kT[h][:, hf * 512:(hf + 1) * 512], psk[:])
        # V: vsb[h][:, (hf*4+kbl)*128 + d] = gathered_x @ wv  [tok, d]
        psv = psV.tile([P, 512], F32)
        for kbl in range(4):
            for c in range(DMC):
                nc.tensor.matmul(
                    psv[:, kbl * P:(kbl + 1) * P],
                    lhsT=xts[:, c, kbl * P:(kbl + 1) * P],
                    rhs=wv_sb[:, c * 512 + h * P: c * 512 + (h + 1) * P],
                    start=(c == 0), stop=(c == DMC - 1))
        nc.vector.tensor_copy(vsb[h][:, hf * 512:(hf + 1) * 512], psv[:])
        if hf == 1:
            # vsum[h] = (1/K) * sum over all selected tokens of v
            pvs = psVS.tile([1, D], F32)
            for kb in range(KB):
                nc.tensor.matmul(
                    pvs[:], lhsT=oinv[:], rhs=vsb[h][:, kb * P:(kb + 1) * P],
                    start=(kb == 0), stop=(kb == KB - 1))
            nc.vector.tensor_copy(vsum[h][:], pvs[:])

    for t in range(TOKC):
        emit_a_chunk(t)
        if t % 2 == 0 and t + 2 < TOKC:
            emit_xa_dmas((t + 2) // 2)
        if t >= 1:
            emit_b_half(t - 1)
        if t + 2 < TOKC:
            issue_gather(t + 2)
    emit_b_half(6)
    emit_b_half(7)
    abstack.close()

    if dbg_d is not None:
        nc.sync.dma_start(dbg_d[:, 0:4096], qT[0][:, :])
        nc.sync.dma_start(dbg_d[:, 4096:5120], kT[0][:, :])
        nc.sync.dma_start(dbg_d[:, 5120:6144], vsb[0][:, :])
        nc.sync.dma_start(dbg_d[0:1, 6144:6272], vsum[0][:, :])

    bstack.close()

    # ---------------- phase C: attention + Wo ----------------
    with tc.tile_pool(name="wop", bufs=1) as wop, \
         tc.tile_pool(name="ptp", bufs=22) as ptp, \
         tc.tile_pool(name="indp", bufs=8) as indp, \
         tc.tile_pool(name="attnp", bufs=8) as attnp, \
         tc.tile_pool(name="rowp", bufs=2) as rowp, \
         tc.tile_pool(name="sbp", bufs=2) as sbp, \
         tc.tile_pool(name="rbcp", bufs=3) as rbcp, \
         tc.tile_pool(name="outp", bufs=4) as outp, \
         tc.tile_pool(name="psL", bufs=3, space="PSUM") as psL, \
         tc.tile_pool(name="psO", bufs=2, space="PSUM") as psO, \
         tc.tile_pool(name="psSum", bufs=1, space="PSUM") as psSum, \
         tc.tile_pool(name="psW", bufs=2, space="PSUM") as psW:
        wo_sb = wop.tile([P, NH * DM], BF16, tag="wo")
        nc.sync.dma_start(wo_sb[:], wo_d[:, :])

        pending_wo = [None]

        def make_wo_parts(qc, attn_t):
            def part(tb):
                osb = outp.tile([P, DM], BF16, tag="osb")
                for n in range(4):
                    pw = psW.tile([P, 512], F32, tag="pw",
                                  name=f"pw{qc}_{tb}_{n}")
                    for hh in range(NH):
                        nc.tensor.matmul(
                            pw[:],
                            lhsT=attn_t[hh][:, tb * P:(tb + 1) * P],
                            rhs=wo_sb[:, hh * DM + n * 512: hh * DM + (n + 1) * 512],
                            start=(hh == 0), stop=(hh == NH - 1))
                    if (tb + n) % 4 == 3:
                        nc.vector.tensor_copy(osb[:, n * 512:(n + 1) * 512], pw[:])
                    else:
                        nc.scalar.copy(osb[:, n * 512:(n + 1) * 512], pw[:])
                nc.sync.dma_start(
                    out_d[qc * 512 + tb * P: qc * 512 + (tb + 1) * P, :],
                    osb[:])
            return [lambda tb=tb: part(tb) for tb in range(4)]

        def wo_part(i):
            if pending_wo[0] is not None:
                pending_wo[0][i]()

        for qc in range(QC):
            sums = psSum.tile([P, 512], F32, tag="sums", name=f"sums{qc}")
            attn_t = {}
            po_t = {}
            use_t = {}
            padd_t = {}
            rrow_t = {}
            rec_bf = {}

            def stage1(h):
                """logits + exp + mask + incremental denominator pre-add"""
                bl = kept[(qc, h)]
                uses = []
                padd = []
                for b in bl:
                    pl = psL.tile([P, 512], F32)
                    nc.tensor.matmul(
                        pl[:],
                        lhsT=kT[h][:, b * P:(b + 1) * P],
                        rhs=qT[h][:, qc * 512:(qc + 1) * 512],
                        start=True, stop=True)
                    pt = ptp.tile([P, 512], BF16, tag="pt")
                    nc.scalar.activation(pt[:], pl[:], AF.Exp)
                    if (qc, h, b) in pcol:
                        col = pcol[(qc, h, b)]
                        ind = indp.tile([P, 512], BF16, tag="ind")
                        nc.vector.tensor_scalar(
                            out=ind[:], in0=iota[:], scalar1=mt[:, col:col + 1],
                            scalar2=None, op0=AL.is_ge)
                        ptm = ptp.tile([P, 512], BF16, tag="pt")
                        nc.vector.tensor_tensor(
                            out=ptm[:], in0=pt[:], in1=ind[:], op=AL.mult)
                        uses.append(ptm)
                        if dbg_d is not None and qc == 4 and h == 0 and b == bl[-1]:
                            nc.sync.dma_start(dbg_d[:, 6784:7296], ind[:])
                    else:
                        uses.append(pt)
                    if dbg_d is not None and qc == 4 and h == 0 and b == bl[-1]:
                        nc.sync.dma_start(dbg_d[:, 6272:6784], uses[-1][:])
                    # pairwise pre-add halves the denominator matmul chain
                    if len(uses) % 2 == 0:
                        pr = ptp.tile([P, 512], BF16, tag="pt")
                        nc.vector.tensor_tensor(
                            out=pr[:], in0=uses[-2][:], in1=uses[-1][:],
                            op=AL.add)
                        padd.append(pr)
                use_t[h] = uses
                if len(uses) % 2 == 1:
                    padd.append(uses[-1])
                padd_t[h] = padd

            def stage2(h):
                """PV + denominator matmul + fix for head h"""
                bl = kept[(qc, h)]
                uses = use_t[h]
                po = psO.tile([P, 512], F32, tag="po", name=f"po{qc}_{h}")
                po_t[h] = po
                if not bl:
                    nc.tensor.matmul(po[:], lhsT=vsum[h][:], rhs=onesrow[:],
                                     start=True, stop=True)
                    return
                need_fix = qc in fixqc
                for j, b in enumerate(bl):
                    nc.tensor.matmul(
                        po[:],
                        lhsT=vsb[h][:, b * P:(b + 1) * P],
                        rhs=uses[j][:],
                        start=(j == 0),
                        stop=(j == len(bl) - 1 and not need_fix))
                srow = sums[32 * h:32 * h + 1, :]
                pads = padd_t[h]
                for i, pr in enumerate(pads):
                    nc.tensor.matmul(srow, lhsT=ones[:], rhs=pr[:],
                                     start=(i == 0), stop=(i == len(pads) - 1),
                                     tile_position=(0, 32 * h))
                if need_fix:
                    fixf = rowp.tile([1, 512], F32, tag="fixf")
                    nc.vector.tensor_scalar(
                        out=fixf[:], in0=srow, scalar1=0.0, scalar2=None,
                        op0=AL.is_equal)
                    fixb = rowp.tile([1, 512], BF16, tag="fixb")
                    nc.vector.tensor_copy(fixb[:], fixf[:])
                    sumb = rowp.tile([1, 512], F32, tag="sumb")
                    nc.vector.tensor_tensor(
                        out=sumb[:], in0=srow, in1=fixf[:], op=AL.add)
                    nc.tensor.matmul(po[:], lhsT=vsum[h][:], rhs=fixb[:],
                                     start=False, stop=True)
                    # per-head reciprocal at partition 0 (fix path only)
                    rsc = rowp.tile([1, 512], F32, tag="rsc")
                    rss = rowp.tile([1, 512], F32, tag="rss")
                    nc.vector.reciprocal_approx_accurate(
                        out=rsc[:], in_=sumb[:], scratch=rss[:])
                    rrow = rowp.tile([1, 512], BF16, tag="rrow")
                    nc.vector.tensor_copy(rrow[:], rsc[:])
                    rrow_t[h] = rrow

            def srecip(half):
                """batched reciprocal for heads 2*half, 2*half+1 (non-fix)"""
                if qc in fixqc:
                    return
                ssb = sbp.tile([64, 512], F32, tag=f"ssb{half}")
                nc.vector.tensor_copy(ssb[:], sums[64 * half:64 * half + 64, :])
                rsc = sbp.tile([64, 512], F32, tag=f"rsc{half}")
                rss = sbp.tile([64, 512], F32, tag=f"rss{half}")
                nc.vector.reciprocal_approx_accurate(
                    out=rsc[:], in_=ssb[:], scratch=rss[:])
                rb = sbp.tile([64, 512], BF16, tag=f"rbf{half}")
                nc.vector.tensor_copy(rb[:], rsc[:])
                rec_bf[half] = rb

            def stage3(h):
                """broadcast reciprocal + normalize head h"""
                po = po_t[h]
                at = attnp.tile([P, 512], BF16, tag="attn", name=f"at{qc}_{h}")
                if not kept[(qc, h)]:
                    nc.vector.tensor_copy(at[:], po[:])
                    attn_t[h] = at
                    return
                pbt = psW.tile([P, 512], F32, tag="pw", name=f"pbt{qc}_{h}")
                if qc in fixqc:
                    nc.tensor.matmul(pbt[:], lhsT=onesall[0:1, :],
                                     rhs=rrow_t[h][:], start=True, stop=True)
                else:
                    ro = 32 * (h % 2)
                    nc.tensor.matmul(
                        pbt[:], lhsT=onesall[ro:ro + 1, :],
                        rhs=rec_bf[h // 2][ro:ro + 1, :],
                        start=True, stop=True, tile_position=(ro, 0))
                rbc = rbcp.tile([P, 512], BF16, tag="rbc")
                nc.vector.tensor_copy(rbc[:], pbt[:])
                nc.vector.tensor_tensor(
                    out=at[:], in0=po[:], in1=rbc[:], op=AL.mult)
                if dbg_d is not None and qc == 4 and h == 0:
                    nc.sync.dma_start(dbg_d[:, 7296:7808], at[:])
                if dbg_d is not None and qc == 1:
                    nc.sync.dma_start(dbg_d[:, 10368 + 512 * h:10368 + 512 * (h + 1)],
                                      at[:])
                attn_t[h] = at

            # pipeline: previous qc's Wo tb-blocks are spread through this
            # qc's early stages; reciprocal batches release po banks in pairs.
            stage1(0)
            wo_part(0)
            stage1(1)
            wo_part(1)
            stage2(0)
            wo_part(2)
            stage1(2)
            stage2(1)
            wo_part(3)
            srecip(0)
            stage3(0)
            stage1(3)
            stage3(1)
            stage2(2)
            stage2(3)
            srecip(1)
            stage3(2)
            stage3(3)

            if dbg_d is not None and qc == 1:
                sdump = outp.tile([P, DM], BF16, tag="osb")
                nc.vector.tensor_copy(sdump[:, 0:512], sums[:])
                nc.sync.dma_start(dbg_d[:, 12416:12928], sdump[:, 0:512])

            pending_wo[0] = make_wo_parts(qc, attn_t)

        for i in range(4):
            wo_part(i)


# ---------------------------------------------------------------------------
# host side
# ---------------------------------------------------------------------------

def make_in_maps(x, Wq, Wk, Wv, Wo, anchor_indices, cls):
    import ml_dtypes
    bf = ml_dtypes.bfloat16
    kept, partial_order, fixqc = cls
    scale = 1.0 / np.sqrt(np.float32(D))
    x = np.asarray(x, dtype=np.float32)
    Wq = np.asarray(Wq, dtype=np.float32)
    Wk = np.asarray(Wk, dtype=np.float32)
    Wv = np.asarray(Wv, dtype=np.float32)
    Wo = np.asarray(Wo, dtype=np.float32)
    tok = _sorted_tokens(anchor_indices)

    xT_b = [np.ascontiguousarray(x[b].T).astype(bf) for b in range(B)]
    xg_b = [np.ascontiguousarray(x[b]).astype(bf) for b in range(B)]

    in_maps = []
    for core in range(8):
        b, hg = core // 4, core % 4
        sl = slice(4 * hg * D, (4 * hg + 4) * D)

        def sbuf_layout(w):
            # [C*128, N] -> [128, C*N]: row p holds chunk-major slices
            cn = w.shape[0] // P
            return np.ascontiguousarray(
                w.reshape(cn, P, w.shape[1]).transpose(1, 0, 2).reshape(P, -1)
            ).astype(bf)

        wq_c = sbuf_layout(Wq[:, sl] * scale)
        wk_c = sbuf_layout(Wk[:, sl])
        wv_c = sbuf_layout(Wv[:, sl])
        wo_c = sbuf_layout(Wo[sl, :])

        # gather indices: per (h, half) group of 512, entry i wrapped to
        # [i % 16, col + i // 16], replicated across the 8 gpsimd stripes
        gidx_c = np.zeros((16, NH * (K // 16)), dtype=np.int16)
        for h in range(NH):
            for hf in range(2):
                seg = tok[core, h, hf * 512:(hf + 1) * 512].astype(np.int16)
                gidx_c[:, h * 64 + hf * 32: h * 64 + (hf + 1) * 32] = \
                    seg.reshape(32, 16).T
        gidx_c = np.tile(gidx_c, (8, 1))

        npart = max(1, len(partial_order))
        mt_c = np.zeros((P, npart), dtype=np.float32)
        for i, (qc, h, bb) in enumerate(partial_order):
            mt_c[:, i] = tok[core, h, bb * P:(bb + 1) * P] - 512.0 * qc - 0.5

        in_maps.append({
            "xT": xT_b[b], "xg": xg_b[b], "wq": wq_c, "wk": wk_c, "wv": wv_c,
            "wo": wo_c, "gidx": gidx_c, "mt": mt_c,
        })
    return in_maps


_NC_CACHE = {}


def get_nc(cls):
    key = (tuple(sorted(cls[0].items())), cls[1], cls[2])
    if key not in _NC_CACHE:
        _NC_CACHE[key] = build_nc(cls)
    return _NC_CACHE[key]


def _ensure_axon_hook_stub():
    # The NTFF profile hook module is absent in some containers; stub it so
    # run_bass_kernel_spmd(trace=True) degrades to a no-trace run.
    import sys, types
    try:
        from antenv import axon_hooks  # noqa: F401
    except ImportError:
        mod = types.ModuleType("antenv.axon_hooks")
        mod.get_axon_ntff_profile_hook = lambda: None
        sys.modules["antenv.axon_hooks"] = mod
        import antenv
        antenv.axon_hooks = mod


def kernel(x, Wq, Wk, Wv, Wo, anchor_indices, _trace=False, _trace_dir=None):
    cls = classify(anchor_indices)
    in_maps = make_in_maps(x, Wq, Wk, Wv, Wo, anchor_indices, cls)
    nc = get_nc(cls)
    if _trace:
        _ensure_axon_hook_stub()
    res = bass_utils.run_bass_kernel_spmd(
        nc, in_maps, core_ids=list(range(8)), trace=_trace, tmpdir=_trace_dir)
    out = np.zeros((B, S, DM), dtype=np.float32)
    for core in range(8):
        out[core // 4] += res.results[core]["out"].astype(np.float32)
    if _trace:
        kernel.last_exec_time_ns = res.exec_time_ns
        kernel.last_results = res
    return out
